# revision 20
# baseline (speedup 1.0000x reference)
"""Bigram LM forward on 8 Trainium2 NeuronCores.

Model (B=2, T=2048, E=1024, H=16, D=64, V=32000):
  x = tok_table[tokens] + pos_emb
  q,k,v = per-head projections; causal attention with softmax/sqrt(D)
  (faithful quirk: scale applied AFTER softmax -> fold 1/8 into Wv)
  logits = concat-heads(o) @ Wo.T + bo ; loss = CE(logits, targets)

Sharding: heads across 8 cores (2 each) for attention; vocab (Wo rows)
across 8 cores (4000 each) for the dominant [4096,1024]x[1024,32000]
projection. One AllGather of the per-core o^T slice [128,4096] between
the two phases. Per-row CE partials (sum exp over the vocab shard) are
computed on-device; host combines partials + extracts target logits.

All matmuls run in float32r (TF32-like: ~2e-4 rel err, bf16-speed).
Everything is computed "transposed" so no on-chip transpose of the
[T,T] probability matrix is ever needed:
  w^T[s,t] = k^T.T q^T directly (operand swap), softmax normalization
  deferred via a ones-column in v_ext (Z row) + per-row reciprocal
  broadcast with a K=1 matmul.
"""

import numpy as np

B, T, E, H, V = 2, 2048, 1024, 16, 32000
D = E // H
NC = 8
BT = B * T                 # 4096 token rows
HL = H // NC               # 2 heads per core
VS = V // NC               # 4000 vocab rows per core
P = 128

_CACHE = {}
_RUN_KW = {}  # test harness may set e.g. {"trace": True}
LAST_RESULT = None


def _build():
    import concourse.bass as bass
    import concourse.mybir as mybir
    import concourse.tile as tile
    from concourse import bacc
    dt = mybir.dt
    F32 = dt.float32
    F32R = dt.float32r
    Exp = mybir.ActivationFunctionType.Exp
    Ident = mybir.ActivationFunctionType.Identity
    add_op = mybir.AluOpType.add
    mult_op = mybir.AluOpType.mult

    nc = bacc.Bacc("TRN2", target_bir_lowering=False, debug=False,
                   enable_asserts=False, num_devices=NC)

    # ---- DRAM parameters (per-core) ----
    tokens = nc.dram_tensor("tokens", [BT, 1], dt.int32, kind="ExternalInput").ap()
    tok_table = nc.dram_tensor("tok_table", [V, E], F32, kind="ExternalInput").ap()
    pos = nc.dram_tensor("pos", [T, E], F32, kind="ExternalInput").ap()
    wqT = nc.dram_tensor("wqT", [E, P], F32R, kind="ExternalInput").ap()
    wkT = nc.dram_tensor("wkT", [E, P], F32R, kind="ExternalInput").ap()
    wvT = nc.dram_tensor("wvT", [E, P], F32R, kind="ExternalInput").ap()
    bqc = nc.dram_tensor("bqc", [P, 1], F32, kind="ExternalInput").ap()
    bkc = nc.dram_tensor("bkc", [P, 1], F32, kind="ExternalInput").ap()
    bvr = nc.dram_tensor("bvr", [P, P], F32, kind="ExternalInput").ap()
    woT = nc.dram_tensor("woT", [E, VS], F32R, kind="ExternalInput").ap()
    bor = nc.dram_tensor("bor", [P, VS], F32, kind="ExternalInput").ap()
    mtri = nc.dram_tensor("mtri", [P, P], F32R, kind="ExternalInput").ap()
    identp = nc.dram_tensor("identp", [P, P], F32R, kind="ExternalInput").ap()
    onesp = nc.dram_tensor("onesp", [P, D], F32R, kind="ExternalInput").ap()

    logits = nc.dram_tensor("logits", [BT, VS], F32, kind="ExternalOutput").ap()
    ce = nc.dram_tensor("ce", [P, BT // P], F32, kind="ExternalOutput").ap()

    TCH = 512                 # t-chunk for phase A
    NA = BT // TCH            # 8
    NT128 = BT // P           # 32 t-chunks of 128
    VCH = 500                 # vocab free chunk (one PSUM bank)
    NV = VS // VCH            # 8

    with tile.TileContext(nc) as tc:
        with (
            tc.tile_pool(name="consts", bufs=1) as cp,
            tc.tile_pool(name="dram", bufs=1, space="DRAM") as dr,
        ):
            # ---- constants ----
            ident = cp.tile([P, P], F32R)
            nc.sync.dma_start(ident[:], identp[:])
            mtri_sb = cp.tile([P, P], F32R)
            nc.sync.dma_start(mtri_sb[:], mtri[:])
            bq_sb = cp.tile([P, 1], F32)
            nc.sync.dma_start(bq_sb[:], bqc[:])
            bk_sb = cp.tile([P, 1], F32)
            nc.sync.dma_start(bk_sb[:], bkc[:])
            bvr_sb = cp.tile([P, P], F32)
            nc.sync.dma_start(bvr_sb[:], bvr[:])
            ones_sb = cp.tile([P, D], F32R)
            nc.sync.dma_start(ones_sb[:], onesp[:])
            wq_sb = cp.tile([P, E // P, P], F32R)
            nc.sync.dma_start(wq_sb[:], wqT.rearrange("(k p) d -> p k d", p=P))
            wk_sb = cp.tile([P, E // P, P], F32R)
            nc.sync.dma_start(wk_sb[:], wkT.rearrange("(k p) d -> p k d", p=P))
            wv_sb = cp.tile([P, E // P, P], F32R)
            nc.sync.dma_start(wv_sb[:], wvT.rearrange("(k p) d -> p k d", p=P))

            # collective buffers
            ag_in = dr.tile([P, BT], F32R)
            ag_out = dr.tile([P * NC, BT], F32R, addr_space="Shared")

            qp_cm = tc.tile_pool(name="qkv", bufs=1)
            qp = qp_cm.__enter__()
            # persistent qkv activations (f32r)
            qT = qp.tile([P, BT], F32R)        # [2 heads x 64 d, t]
            kT = qp.tile([P, BT], F32R)
            # v_ext: [t-part, t-chunk, 2 x (64 d + ones col)]
            vx = qp.tile([P, NT128, 2 * (D + 1)], F32R)
            nc.gpsimd.memset(vx[:, :, D:D + 1].bitcast(F32), 1.0)
            nc.gpsimd.memset(vx[:, :, 2 * D + 1:2 * D + 2].bitcast(F32), 1.0)

            # ================= Phase A: embed -> xT -> q,k,v =================
            with (
                tc.tile_pool(name="pa", bufs=2) as pa,
                tc.tile_pool(name="pa_ps", bufs=3, space="PSUM") as pa_ps,
                tc.tile_pool(name="pa_ps2", bufs=3, space="PSUM") as pa_ps2,
            ):
                for a in range(NA):
                    xT = pa.tile([P, E // P, TCH], F32R, tag="xT")
                    for j in range(TCH // P):
                        t128 = a * (TCH // P) + j
                        idx = pa.tile([P, 1], dt.int32, tag="idx")
                        nc.sync.dma_start(idx[:], tokens[t128 * P:(t128 + 1) * P, :])
                        xg = pa.tile([P, E], F32, tag="xg")
                        nc.gpsimd.indirect_dma_start(
                            out=xg[:], out_offset=None, in_=tok_table[:],
                            in_offset=bass.IndirectOffsetOnAxis(ap=idx[:, :1], axis=0),
                        )
                        xp = pa.tile([P, E], F32, tag="xp")
                        tt = (t128 % (T // P)) * P
                        nc.sync.dma_start(xp[:], pos[tt:tt + P, :])
                        xs = pa.tile([P, E], F32R, tag="xs")
                        nc.vector.tensor_tensor(out=xs[:], in0=xg[:], in1=xp[:], op=add_op)
                        for e8 in range(E // P):
                            pst = pa_ps.tile([P, P], F32R, tag="pst")
                            nc.tensor.transpose(pst[:], xs[:, e8 * P:(e8 + 1) * P], ident[:])
                            nc.scalar.copy(xT[:, e8, j * P:(j + 1) * P], pst[:])
                    # q,k (transposed layout) + bias on ACT
                    for wsb, b_sb, dst in ((wq_sb, bq_sb, qT), (wk_sb, bk_sb, kT)):
                        psq = pa_ps2.tile([P, TCH], F32, tag="psq")
                        for e8 in range(E // P):
                            nc.tensor.matmul(psq[:], wsb[:, e8, :], xT[:, e8, :],
                                             start=(e8 == 0), stop=(e8 == E // P - 1))
                        nc.scalar.activation(dst[:, a * TCH:(a + 1) * TCH], psq[:],
                                             Ident, bias=b_sb[:])
                    # vT then transpose to v natural (+ bias replicated)
                    psv = pa_ps2.tile([P, TCH], F32, tag="psq")
                    for e8 in range(E // P):
                        nc.tensor.matmul(psv[:], wv_sb[:, e8, :], xT[:, e8, :],
                                         start=(e8 == 0), stop=(e8 == E // P - 1))
                    vTc = pa.tile([P, TCH], F32R, tag="vTc")
                    nc.vector.tensor_copy(out=vTc[:], in_=psv[:])
                    for j in range(TCH // P):
                        t128 = a * (TCH // P) + j
                        psvt = pa_ps.tile([P, P], F32R, tag="pst")
                        nc.tensor.transpose(psvt[:], vTc[:, j * P:(j + 1) * P], ident[:])
                        for h in range(2):
                            nc.vector.tensor_tensor(
                                out=vx[:, t128, h * (D + 1):h * (D + 1) + D],
                                in0=psvt[:, h * D:(h + 1) * D],
                                in1=bvr_sb[:, h * D:(h + 1) * D], op=add_op)

            # ================= Phase B: attention per (b, h) =================
            NS = T // P              # 16 s-chunks of 128
            NQ = T // 512            # 4 t-chunks of 512
            with (
                tc.tile_pool(name="pb", bufs=1) as pb,
                tc.tile_pool(name="pb2", bufs=2) as pb2,
                tc.tile_pool(name="pb_ps", bufs=2, space="PSUM") as pb_ps,
                tc.tile_pool(name="pb_ps2", bufs=2, space="PSUM") as pb_ps2,
                tc.tile_pool(name="pb_ps3", bufs=2, space="PSUM") as pb_ps3,
            ):
                for b in range(B):
                    for h in range(2):
                        qs = qT[h * D:(h + 1) * D, b * T:(b + 1) * T]
                        ks = kT[h * D:(h + 1) * D, b * T:(b + 1) * T]
                        pts = []
                        for si in range(NS):
                            toff = (si // 4) * 512
                            pt = pb.tile([P, T - toff], F32R, tag=f"pt{si}")
                            pts.append(pt)
                            m = si % 4
                            for tj in range(si // 4, NQ):
                                psw = pb_ps.tile([P, 512], F32, tag="psw")
                                nc.tensor.matmul(psw[:], ks[:, si * P:(si + 1) * P],
                                                 qs[:, tj * 512:(tj + 1) * 512],
                                                 start=True, stop=True)
                                nc.scalar.activation(
                                    pt[:, tj * 512 - toff:(tj + 1) * 512 - toff],
                                    psw[:], Exp)
                            if m > 0:
                                nc.gpsimd.memset(pt[:, 0:P * m].bitcast(F32), 0.0)
                            nc.vector.tensor_tensor(out=pt[:, P * m:P * (m + 1)],
                                                    in0=pt[:, P * m:P * (m + 1)],
                                                    in1=mtri_sb[:], op=mult_op)
                        for tq in range(NQ):
                            pso = pb_ps2.tile([D + 1, 512], F32, tag="pso")
                            nsi = 4 * tq + 4
                            for si in range(nsi):
                                toff = (si // 4) * 512
                                nc.tensor.matmul(
                                    pso[:],
                                    vx[:, b * NS + si, h * (D + 1):(h + 1) * (D + 1)],
                                    pts[si][:, tq * 512 - toff:(tq + 1) * 512 - toff],
                                    start=(si == 0), stop=(si == nsi - 1))
                            rr = pb2.tile([P, 512], F32R, tag="rr")
                            with nc.allow_low_precision(reason="1/Z at f32r feeds f32r broadcast matmul"):
                                nc.vector.reciprocal(rr[D:D + 1, :], pso[D:D + 1, :])
                            psr = pb_ps3.tile([D, 512], F32, tag="psr")
                            nc.tensor.matmul(psr[:], ones_sb[D:D + 1, :],
                                             rr[D:D + 1, :], start=True, stop=True)
                            rrep = pb2.tile([D, 512], F32R, tag="rrep")
                            nc.scalar.copy(rrep[:], psr[:])
                            ot = pb2.tile([D, 512], F32R, tag="ot")
                            nc.vector.tensor_tensor(out=ot[:], in0=pso[:D, :],
                                                    in1=rrep[:], op=mult_op)
                            nc.sync.dma_start(
                                ag_in[h * D:(h + 1) * D,
                                      b * T + tq * 512:b * T + (tq + 1) * 512],
                                ot[:])

            qp_cm.__exit__(None, None, None)

            # ================= AllGather =================
            nc.gpsimd.collective_compute(
                "AllGather", mybir.AluOpType.bypass,
                replica_groups=[list(range(NC))],
                ins=[ag_in.opt()], outs=[ag_out.opt()],
            )

            # ================= Phase C: vocab projection + CE =================
            with (
                tc.tile_pool(name="pc", bufs=1) as pc,
                tc.tile_pool(name="pc2", bufs=2) as pc2,
                tc.tile_pool(name="pc3", bufs=2) as pc3,
                tc.tile_pool(name="pc_ps", bufs=1, space="PSUM") as pc_ps,
            ):
                woS = pc.tile([P, E // P, VS], F32R)
                nc.sync.dma_start(woS[:], woT.rearrange("(k p) v -> p k v", p=P))
                bon = pc.tile([P, VS], F32)
                nc.sync.dma_start(bon[:], bor[:])
                ce_acc = pc.tile([P, NT128, NV], F32)
                ce_sb = pc.tile([P, NT128], F32)
                ag_ap = ag_out.opt().rearrange("(k p) t -> p k t", p=P)
                for mI in range(NT128):
                    otm = pc2.tile([P, E // P, P], F32R, tag="otm")
                    nc.sync.dma_start(otm[:], ag_ap[:, :, mI * P:(mI + 1) * P])
                    psls = [pc_ps.tile([P, VCH], F32, tag=f"psl{n}", name=f"psl{n}")
                            for n in range(NV)]
                    for e8 in range(E // P):
                        for n in range(NV):
                            nc.tensor.matmul(psls[n][:],
                                             otm[:, e8, :],
                                             woS[:, e8, n * VCH:(n + 1) * VCH],
                                             start=(e8 == 0), stop=(e8 == E // P - 1))
                    for n in range(NV):
                        lsb = pc3.tile([P, VCH], F32, tag="lsb")
                        nc.vector.tensor_tensor(out=lsb[:], in0=psls[n][:],
                                                in1=bon[:, n * VCH:(n + 1) * VCH],
                                                op=add_op)
                        nc.sync.dma_start(
                            logits[mI * P:(mI + 1) * P, n * VCH:(n + 1) * VCH], lsb[:])
                        esc = pc3.tile([P, VCH], F32, tag="esc")
                        nc.scalar.activation(esc[:], lsb[:], Exp,
                                             accum_out=ce_acc[:, mI, n:n + 1])
                for mI in range(NT128):
                    nc.vector.reduce_sum(out=ce_sb[:, mI:mI + 1], in_=ce_acc[:, mI, :],
                                         axis=mybir.AxisListType.X)
                nc.sync.dma_start(ce[:], ce_sb[:])

    nc.compile()
    return nc


def _get_nc():
    if "nc" not in _CACHE:
        _CACHE["nc"] = _build()
    return _CACHE["nc"]


def kernel(tokens, targets, tok_table, pos_emb, Wq, bq, Wk, bk, Wv, bv, Wo, bo):
    from concourse.bass_utils import run_bass_kernel_spmd

    tokens = np.asarray(tokens)
    targets = np.asarray(targets)
    tok_table = np.ascontiguousarray(np.asarray(tok_table, np.float32))
    pos_emb = np.ascontiguousarray(np.asarray(pos_emb, np.float32))
    Wq = np.asarray(Wq, np.float32)
    Wk = np.asarray(Wk, np.float32)
    Wv = np.asarray(Wv, np.float32)
    bq = np.asarray(bq, np.float32)
    bk = np.asarray(bk, np.float32)
    bv = np.asarray(bv, np.float32)
    Wo = np.asarray(Wo, np.float32)
    bo = np.asarray(bo, np.float32)

    tok_i = tokens.reshape(BT, 1).astype(np.int32)
    scale = np.float32(1.0 / np.sqrt(D))  # folded into Wv/bv (quirk: post-softmax)

    # host-side layout prep (pure reshapes/transposes of weights)
    mtri = np.triu(np.ones((P, P), np.float32))  # [s,t] valid iff t >= s
    in_maps = []
    for c in range(NC):
        h0 = HL * c
        wq_c = np.ascontiguousarray(
            Wq[h0:h0 + HL].reshape(HL * D, E).T)            # [E, 128]
        wk_c = np.ascontiguousarray(Wk[h0:h0 + HL].reshape(HL * D, E).T)
        wv_c = np.ascontiguousarray((Wv[h0:h0 + HL] * scale).reshape(HL * D, E).T)
        bq_c = bq[h0:h0 + HL].reshape(P, 1).copy()
        bk_c = bk[h0:h0 + HL].reshape(P, 1).copy()
        bv_c = np.broadcast_to((bv[h0:h0 + HL] * scale).reshape(1, P), (P, P)).copy()
        wo_c = np.ascontiguousarray(Wo[c * VS:(c + 1) * VS].T)  # [E, VS]
        bo_c = np.broadcast_to(bo[c * VS:(c + 1) * VS][None, :], (P, VS)).copy()
        in_maps.append({
            "tokens": tok_i, "tok_table": tok_table, "pos": pos_emb,
            "wqT": wq_c, "wkT": wk_c, "wvT": wv_c,
            "bqc": bq_c, "bkc": bk_c, "bvr": bv_c,
            "woT": wo_c, "bor": bo_c, "mtri": mtri,
            "identp": np.eye(P, dtype=np.float32),
            "onesp": np.ones((P, D), np.float32),
        })

    nc = _get_nc()
    res = run_bass_kernel_spmd(nc, in_maps, core_ids=list(range(NC)), **_RUN_KW)
    global LAST_RESULT
    LAST_RESULT = res

    logits_full = np.empty((BT, V), np.float32)
    S = np.zeros(BT, np.float64)
    for c in range(NC):
        r = res.results[c]
        logits_full[:, c * VS:(c + 1) * VS] = r["logits"]
        S += r["ce"].T.reshape(BT).astype(np.float64)

    tl = targets.reshape(BT).astype(np.int64)
    l_tgt = logits_full[np.arange(BT), tl].astype(np.float64)
    loss = np.float32(np.mean(np.log(S) - l_tgt))
    return logits_full, loss


# revision 23
# speedup vs baseline: 1.0735x; 1.0735x over previous
"""Bigram LM forward on 8 Trainium2 NeuronCores.

Model (B=2, T=2048, E=1024, H=16, D=64, V=32000):
  x = tok_table[tokens] + pos_emb
  q,k,v = per-head projections; causal attention with softmax/sqrt(D)
  (faithful quirk: scale applied AFTER softmax -> fold 1/8 into Wv)
  logits = concat-heads(o) @ Wo.T + bo ; loss = CE(logits, targets)

Sharding: heads across 8 cores (2 each) for attention; vocab (Wo rows)
across 8 cores (4000 each) for the dominant [4096,1024]x[1024,32000]
projection. Two AllGathers (one per batch element) of the per-core o^T
slice move [128,2048] -> [1024,2048] between the phases, overlapping
the second with batch-1 attention. Per-row CE partials (sum of exp over
the vocab shard) are computed on-device; the host combines partials and
extracts target logits.

All matmuls run in float32r (TF32-like: ~2e-4 rel err, bf16-rate).
Everything is computed "transposed" so no on-chip transpose of the
[T,T] probability matrix is ever needed:
  w^T[s,t] = k^T.T q^T directly (operand swap), softmax normalization
  deferred via a ones-column in v_ext (Z row) + per-row reciprocal
  broadcast with a K=1 matmul. Consecutive matmuls never repeat the
  same stationary operand (measured 9x slowdown when they do).
"""

import numpy as np

B, T, E, H, V = 2, 2048, 1024, 16, 32000
D = E // H
NC = 8
BT = B * T                 # 4096 token rows
HL = H // NC               # 2 heads per core
VS = V // NC               # 4000 vocab rows per core
P = 128

_CACHE = {}
_RUN_KW = {}  # test harness may set e.g. {"trace": True}
LAST_RESULT = None


def _build():
    import concourse.bass as bass
    import concourse.mybir as mybir
    import concourse.tile as tile
    from concourse import bacc

    dt = mybir.dt
    F32 = dt.float32
    F32R = dt.float32r
    Exp = mybir.ActivationFunctionType.Exp
    Ident = mybir.ActivationFunctionType.Identity
    add_op = mybir.AluOpType.add
    mult_op = mybir.AluOpType.mult

    nc = bacc.Bacc("TRN2", target_bir_lowering=False, debug=False,
                   enable_asserts=False, num_devices=NC)

    # ---- DRAM parameters (per-core) ----
    tokens = nc.dram_tensor("tokens", [BT, 1], dt.int32, kind="ExternalInput").ap()
    tok_table = nc.dram_tensor("tok_table", [V, E], F32, kind="ExternalInput").ap()
    pos = nc.dram_tensor("pos", [T, E], F32, kind="ExternalInput").ap()
    wqT = nc.dram_tensor("wqT", [E, P], F32R, kind="ExternalInput").ap()
    wkT = nc.dram_tensor("wkT", [E, P], F32R, kind="ExternalInput").ap()
    wvT = nc.dram_tensor("wvT", [E, P], F32R, kind="ExternalInput").ap()
    bqc = nc.dram_tensor("bqc", [P, 1], F32, kind="ExternalInput").ap()
    bkc = nc.dram_tensor("bkc", [P, 1], F32, kind="ExternalInput").ap()
    bvr = nc.dram_tensor("bvr", [P, P], F32, kind="ExternalInput").ap()
    woT = nc.dram_tensor("woT", [E, VS], F32R, kind="ExternalInput").ap()
    bor = nc.dram_tensor("bor", [P, VS], F32, kind="ExternalInput").ap()
    mtri = nc.dram_tensor("mtri", [P, P], F32R, kind="ExternalInput").ap()
    identp = nc.dram_tensor("identp", [P, P], F32R, kind="ExternalInput").ap()
    onesp = nc.dram_tensor("onesp", [P, D], F32R, kind="ExternalInput").ap()

    logits = nc.dram_tensor("logits", [BT, VS], F32, kind="ExternalOutput").ap()
    ce = nc.dram_tensor("ce", [P, BT // P], F32, kind="ExternalOutput").ap()

    TCH = 512                 # t-chunk for phase A
    NA = BT // TCH            # 8
    NT128 = BT // P           # 32 t-chunks of 128
    VCH = 500                 # vocab free chunk (one PSUM bank)
    NV = VS // VCH            # 8
    NS = T // P               # 16 s-chunks per (b,h)
    NQ = T // 512             # 4 oT t-chunks per (b,h)

    with tile.TileContext(nc) as tc:
        with (
            tc.tile_pool(name="consts", bufs=1) as cp,
            tc.tile_pool(name="dram", bufs=1, space="DRAM") as dr,
        ):
            # ---- constants ----
            ident = cp.tile([P, P], F32R)
            nc.sync.dma_start(ident[:], identp[:])
            mtri_sb = cp.tile([P, P], F32R)
            nc.sync.dma_start(mtri_sb[:], mtri[:])
            bq_sb = cp.tile([P, 1], F32)
            nc.sync.dma_start(bq_sb[:], bqc[:])
            bk_sb = cp.tile([P, 1], F32)
            nc.sync.dma_start(bk_sb[:], bkc[:])
            bvr_sb = cp.tile([P, P], F32)
            nc.sync.dma_start(bvr_sb[:], bvr[:])
            ones_sb = cp.tile([P, D], F32R)
            nc.sync.dma_start(ones_sb[:], onesp[:])
            wq_sb = cp.tile([P, E // P, P], F32R)
            nc.sync.dma_start(wq_sb[:], wqT.rearrange("(k p) d -> p k d", p=P))
            wk_sb = cp.tile([P, E // P, P], F32R)
            nc.sync.dma_start(wk_sb[:], wkT.rearrange("(k p) d -> p k d", p=P))
            wv_sb = cp.tile([P, E // P, P], F32R)
            nc.sync.dma_start(wv_sb[:], wvT.rearrange("(k p) d -> p k d", p=P))

            # collective buffers (one AllGather per batch element)
            ag_in = [dr.tile([P, T], F32R, name=f"agin{b}") for b in range(B)]
            ag_out = [dr.tile([P * NC, T], F32R, addr_space="Shared",
                              name=f"agout{b}") for b in range(B)]

            qp_cm = tc.tile_pool(name="qkv", bufs=1)
            qp = qp_cm.__enter__()
            # persistent qkv activations (f32r)
            qT = qp.tile([P, BT], F32R)        # [2 heads x 64 d, t]
            kT = qp.tile([P, BT], F32R)
            # v_ext: [t-part, t-chunk, 2 x (64 d + ones col)]
            vx = qp.tile([P, NT128, 2 * (D + 1)], F32R)
            nc.gpsimd.memset(vx[:, :, D:D + 1].bitcast(F32), 1.0)
            nc.gpsimd.memset(vx[:, :, 2 * D + 1:2 * D + 2].bitcast(F32), 1.0)

            # ============ Phases A (embed/qkv) + B (attention) ============
            # pools open together so the scheduler can overlap them
            with (
                tc.tile_pool(name="pa", bufs=2) as pa,
                tc.tile_pool(name="pb", bufs=4) as pb,
                tc.tile_pool(name="pb2", bufs=2) as pb2,
                tc.tile_pool(name="pa_ps", bufs=2, space="PSUM") as pa_ps,
                tc.tile_pool(name="ps512", bufs=2, space="PSUM") as ps512,
                tc.tile_pool(name="pb_ps2", bufs=1, space="PSUM") as pb_ps2,
            ):
                # ---------------- Phase A ----------------
                for a in range(NA):
                    xT = pa.tile([P, E // P, TCH], F32R, tag="xT")
                    for j in range(TCH // P):
                        t128 = a * (TCH // P) + j
                        idx = pa.tile([P, 1], dt.int32, tag="idx")
                        nc.sync.dma_start(idx[:], tokens[t128 * P:(t128 + 1) * P, :])
                        xg = pa.tile([P, E], F32, tag="xg")
                        nc.gpsimd.indirect_dma_start(
                            out=xg[:], out_offset=None, in_=tok_table[:],
                            in_offset=bass.IndirectOffsetOnAxis(ap=idx[:, :1], axis=0),
                        )
                        xp = pa.tile([P, E], F32, tag="xp")
                        tt = (t128 % (T // P)) * P
                        nc.sync.dma_start(xp[:], pos[tt:tt + P, :])
                        xs = pa.tile([P, E], F32R, tag="xs")
                        nc.vector.tensor_tensor(out=xs[:], in0=xg[:], in1=xp[:], op=add_op)
                        for e8 in range(E // P):
                            pst = pa_ps.tile([P, P], F32R, tag="pst")
                            nc.tensor.transpose(pst[:], xs[:, e8 * P:(e8 + 1) * P], ident[:])
                            if e8 % 2 == 0:
                                nc.scalar.copy(xT[:, e8, j * P:(j + 1) * P], pst[:])
                            else:
                                nc.vector.tensor_copy(xT[:, e8, j * P:(j + 1) * P], pst[:])
                    # q,k (transposed layout) + bias on ACT
                    for wsb, b_sb, dst in ((wq_sb, bq_sb, qT), (wk_sb, bk_sb, kT)):
                        psq = ps512.tile([P, TCH], F32, tag="ps512", name="psq")
                        for e8 in range(E // P):
                            nc.tensor.matmul(psq[:], wsb[:, e8, :], xT[:, e8, :],
                                             start=(e8 == 0), stop=(e8 == E // P - 1))
                        nc.scalar.activation(dst[:, a * TCH:(a + 1) * TCH], psq[:],
                                             Ident, bias=b_sb[:])
                    # vT then transpose to v natural (+ bias replicated)
                    psv = ps512.tile([P, TCH], F32, tag="ps512", name="psv")
                    for e8 in range(E // P):
                        nc.tensor.matmul(psv[:], wv_sb[:, e8, :], xT[:, e8, :],
                                         start=(e8 == 0), stop=(e8 == E // P - 1))
                    vTc = pa.tile([P, TCH], F32R, tag="vTc")
                    nc.vector.tensor_copy(out=vTc[:], in_=psv[:])
                    for j in range(TCH // P):
                        t128 = a * (TCH // P) + j
                        psvt = pa_ps.tile([P, P], F32R, tag="pst")
                        nc.tensor.transpose(psvt[:], vTc[:, j * P:(j + 1) * P], ident[:])
                        for h in range(2):
                            nc.vector.tensor_tensor(
                                out=vx[:, t128, h * (D + 1):h * (D + 1) + D],
                                in0=psvt[:, h * D:(h + 1) * D],
                                in1=bvr_sb[:, h * D:(h + 1) * D], op=add_op)

                # ---------------- Phase B ----------------
                # si processed in pairs so consecutive matmuls alternate
                # their stationary operand; oT accumulates in 4 PSUM banks
                # (one per 512-wide t-chunk) so p tiles die right away.
                for b in range(B):
                    for h in range(2):
                        qs = qT[h * D:(h + 1) * D, b * T:(b + 1) * T]
                        ks = kT[h * D:(h + 1) * D, b * T:(b + 1) * T]
                        psos = [pb_ps2.tile([D + 1, 512], F32, tag=f"pso{tq}",
                                            name=f"pso{tq}") for tq in range(NQ)]
                        for sb_ in range(0, NS, 2):
                            pair = (sb_, sb_ + 1)
                            pts = {}
                            # scores + exp (interleave the two si's)
                            for tj in range(sb_ // 4, NQ):
                                for si in pair:
                                    if tj < si // 4:
                                        continue
                                    if si not in pts:
                                        pts[si] = pb.tile([P, T], F32R, tag="pt",
                                                          name=f"pt{si}")
                                    psw = ps512.tile([P, 512], F32, tag="ps512", name="psw")
                                    nc.tensor.matmul(
                                        psw[:], ks[:, si * P:(si + 1) * P],
                                        qs[:, tj * 512:(tj + 1) * 512],
                                        start=True, stop=True)
                                    nc.scalar.activation(
                                        pts[si][:, tj * 512:(tj + 1) * 512],
                                        psw[:], Exp)
                            # mask the diagonal block / zero invalid prefix
                            for si in pair:
                                m = si % 4
                                toff = (si // 4) * 512
                                if m > 0:
                                    nc.gpsimd.memset(
                                        pts[si][:, toff:toff + P * m].bitcast(F32), 0.0)
                                nc.vector.tensor_tensor(
                                    out=pts[si][:, toff + P * m:toff + P * (m + 1)],
                                    in0=pts[si][:, toff + P * m:toff + P * (m + 1)],
                                    in1=mtri_sb[:], op=mult_op)
                            # oT accumulation (alternating lhsT between the pair)
                            for tq in range(sb_ // 4, NQ):
                                for si in pair:
                                    nc.tensor.matmul(
                                        psos[tq][:],
                                        vx[:, b * NS + si, h * (D + 1):(h + 1) * (D + 1)],
                                        pts[si][:, tq * 512:(tq + 1) * 512],
                                        start=(si == 0), stop=(si == 4 * tq + 3))
                            # tq = sb_//4 is complete once pair (4tq+2, 4tq+3) done
                            if sb_ % 4 == 2:
                                tq = sb_ // 4
                                pso = psos[tq]
                                rr = pb2.tile([P, 512], F32R, tag="rr")
                                with nc.allow_low_precision(reason="1/Z feeds f32r bcast"):
                                    nc.vector.reciprocal(rr[D:D + 1, :], pso[D:D + 1, :])
                                psr = ps512.tile([D, 512], F32, tag="ps512", name="psr")
                                nc.tensor.matmul(psr[:D], ones_sb[D:D + 1, :],
                                                 rr[D:D + 1, :], start=True, stop=True)
                                rrep = pb2.tile([D, 512], F32R, tag="rrep")
                                nc.scalar.copy(rrep[:], psr[:D])
                                ot = pb2.tile([D, 512], F32R, tag="ot")
                                nc.vector.tensor_tensor(out=ot[:], in0=pso[:D, :],
                                                        in1=rrep[:], op=mult_op)
                                nc.sync.dma_start(
                                    ag_in[b][h * D:(h + 1) * D,
                                             tq * 512:(tq + 1) * 512],
                                    ot[:])
                    # batch b fully produced -> fire its AllGather
                    nc.gpsimd.collective_compute(
                        "AllGather", mybir.AluOpType.bypass,
                        replica_groups=[list(range(NC))],
                        ins=[ag_in[b].opt()], outs=[ag_out[b].opt()],
                    )

            qp_cm.__exit__(None, None, None)

            # ================= Phase C: vocab projection + CE =================
            with (
                tc.tile_pool(name="pc", bufs=1) as pc,
                tc.tile_pool(name="pc2", bufs=2) as pc2,
                tc.tile_pool(name="pc3", bufs=3) as pc3,
                tc.tile_pool(name="pc_ps", bufs=1, space="PSUM") as pc_ps,
            ):
                woS = pc.tile([P, E // P, VS], F32R)
                nc.sync.dma_start(woS[:], woT.rearrange("(k p) v -> p k v", p=P))
                bon = pc.tile([P, VS], F32)
                nc.sync.dma_start(bon[:], bor[:])
                ce_acc = pc.tile([P, NT128, NV], F32)
                ce_sb = pc.tile([P, NT128], F32)
                for bI in range(B):
                    ag_ap = ag_out[bI].opt().rearrange("(k p) t -> p k t", p=P)
                    for mL in range(T // P):
                        mI = bI * (T // P) + mL
                        otm = pc2.tile([P, E // P, P], F32R, tag="otm")
                        nc.sync.dma_start(otm[:], ag_ap[:, :, mL * P:(mL + 1) * P])
                        psls = [pc_ps.tile([P, VCH], F32, tag=f"psl{n}", name=f"psl{n}")
                                for n in range(NV)]
                        for n in range(NV):
                            for e8 in range(E // P):
                                nc.tensor.matmul(psls[n][:],
                                                 otm[:, e8, :],
                                                 woS[:, e8, n * VCH:(n + 1) * VCH],
                                                 start=(e8 == 0), stop=(e8 == E // P - 1))
                        for n in range(NV):
                            lsb = pc3.tile([P, VCH], F32, tag="lsb")
                            nc.vector.tensor_tensor(out=lsb[:], in0=psls[n][:],
                                                    in1=bon[:, n * VCH:(n + 1) * VCH],
                                                    op=add_op)
                            nc.sync.dma_start(
                                logits[mI * P:(mI + 1) * P, n * VCH:(n + 1) * VCH],
                                lsb[:])
                            esc = pc3.tile([P, VCH], F32, tag="esc")
                            nc.scalar.activation(esc[:], lsb[:], Exp,
                                                 accum_out=ce_acc[:, mI, n:n + 1])
                for mI in range(NT128):
                    nc.vector.reduce_sum(out=ce_sb[:, mI:mI + 1], in_=ce_acc[:, mI, :],
                                         axis=mybir.AxisListType.X)
                nc.sync.dma_start(ce[:], ce_sb[:])

    nc.compile()
    return nc


def _get_nc():
    if "nc" not in _CACHE:
        _CACHE["nc"] = _build()
    return _CACHE["nc"]


def kernel(tokens, targets, tok_table, pos_emb, Wq, bq, Wk, bk, Wv, bv, Wo, bo):
    from concourse.bass_utils import run_bass_kernel_spmd

    tokens = np.asarray(tokens)
    targets = np.asarray(targets)
    tok_table = np.ascontiguousarray(np.asarray(tok_table, np.float32))
    pos_emb = np.ascontiguousarray(np.asarray(pos_emb, np.float32))
    Wq = np.asarray(Wq, np.float32)
    Wk = np.asarray(Wk, np.float32)
    Wv = np.asarray(Wv, np.float32)
    bq = np.asarray(bq, np.float32)
    bk = np.asarray(bk, np.float32)
    bv = np.asarray(bv, np.float32)
    Wo = np.asarray(Wo, np.float32)
    bo = np.asarray(bo, np.float32)

    tok_i = tokens.reshape(BT, 1).astype(np.int32)
    scale = np.float32(1.0 / np.sqrt(D))  # folded into Wv/bv (quirk: post-softmax)

    # host-side layout prep (pure reshapes/transposes of weights)
    mtri = np.triu(np.ones((P, P), np.float32))  # [s,t] valid iff t >= s
    in_maps = []
    for c in range(NC):
        h0 = HL * c
        wq_c = np.ascontiguousarray(
            Wq[h0:h0 + HL].reshape(HL * D, E).T)            # [E, 128]
        wk_c = np.ascontiguousarray(Wk[h0:h0 + HL].reshape(HL * D, E).T)
        wv_c = np.ascontiguousarray((Wv[h0:h0 + HL] * scale).reshape(HL * D, E).T)
        bq_c = bq[h0:h0 + HL].reshape(P, 1).copy()
        bk_c = bk[h0:h0 + HL].reshape(P, 1).copy()
        bv_c = np.broadcast_to((bv[h0:h0 + HL] * scale).reshape(1, P), (P, P)).copy()
        wo_c = np.ascontiguousarray(Wo[c * VS:(c + 1) * VS].T)  # [E, VS]
        bo_c = np.broadcast_to(bo[c * VS:(c + 1) * VS][None, :], (P, VS)).copy()
        in_maps.append({
            "tokens": tok_i, "tok_table": tok_table, "pos": pos_emb,
            "wqT": wq_c, "wkT": wk_c, "wvT": wv_c,
            "bqc": bq_c, "bkc": bk_c, "bvr": bv_c,
            "woT": wo_c, "bor": bo_c, "mtri": mtri,
            "identp": np.eye(P, dtype=np.float32),
            "onesp": np.ones((P, D), np.float32),
        })

    nc = _get_nc()
    res = run_bass_kernel_spmd(nc, in_maps, core_ids=list(range(NC)), **_RUN_KW)
    global LAST_RESULT
    LAST_RESULT = res

    logits_full = np.empty((BT, V), np.float32)
    S = np.zeros(BT, np.float64)
    for c in range(NC):
        r = res.results[c]
        logits_full[:, c * VS:(c + 1) * VS] = r["logits"]
        S += r["ce"].T.reshape(BT).astype(np.float64)

    tl = targets.reshape(BT).astype(np.int64)
    l_tgt = logits_full[np.arange(BT), tl].astype(np.float64)
    loss = np.float32(np.mean(np.log(S) - l_tgt))
    return logits_full, loss


# revision 28
# speedup vs baseline: 1.0927x; 1.0179x over previous
"""Bigram LM forward on 8 Trainium2 NeuronCores.

Model (B=2, T=2048, E=1024, H=16, D=64, V=32000):
  x = tok_table[tokens] + pos_emb
  q,k,v = per-head projections; causal attention with softmax/sqrt(D)
  (faithful quirk: scale applied AFTER softmax -> fold 1/8 into Wv)
  logits = concat-heads(o) @ Wo.T + bo ; loss = CE(logits, targets)

Sharding: heads across 8 cores (2 each) for attention; vocab (Wo rows)
across 8 cores (4000 each) for the dominant [4096,1024]x[1024,32000]
projection. Eight chunked AllGathers (one per (batch, 512-token
quarter)) move each finished o^T chunk [128,512] -> [1024,512] as soon
as both local heads produced it, so only the last chunk's latency is
exposed. Wo^T streams into SBUF in two halves, the first prefetched
during attention. Per-row CE partials (sum of exp over the vocab
shard) are computed on-device; the host combines partials and
extracts target logits.

All matmuls run in float32r (TF32-like: ~2e-4 rel err, bf16-rate).
Everything is computed "transposed" so no on-chip transpose of the
[T,T] probability matrix is ever needed:
  w^T[s,t] = k^T.T q^T directly (operand swap), softmax normalization
  deferred via a ones-column in v_ext (Z row) + per-row reciprocal
  broadcast with a K=1 matmul. Consecutive matmuls never repeat the
  same stationary operand (measured 9x slowdown when they do).
"""

import numpy as np

B, T, E, H, V = 2, 2048, 1024, 16, 32000
D = E // H
NC = 8
BT = B * T                 # 4096 token rows
HL = H // NC               # 2 heads per core
VS = V // NC               # 4000 vocab rows per core
P = 128

_CACHE = {}
_RUN_KW = {}  # test harness may set e.g. {"trace": True}
LAST_RESULT = None


def _build():
    import concourse.bass as bass
    import concourse.mybir as mybir
    import concourse.tile as tile
    from concourse import bacc

    dt = mybir.dt
    F32 = dt.float32
    F32R = dt.float32r
    Exp = mybir.ActivationFunctionType.Exp
    Ident = mybir.ActivationFunctionType.Identity
    add_op = mybir.AluOpType.add
    mult_op = mybir.AluOpType.mult

    nc = bacc.Bacc("TRN2", target_bir_lowering=False, debug=False,
                   enable_asserts=False, num_devices=NC)

    # ---- DRAM parameters (per-core) ----
    tokens = nc.dram_tensor("tokens", [BT, 1], dt.int32, kind="ExternalInput").ap()
    tok_table = nc.dram_tensor("tok_table", [V, E], F32, kind="ExternalInput").ap()
    pos = nc.dram_tensor("pos", [T, E], F32, kind="ExternalInput").ap()
    wqT = nc.dram_tensor("wqT", [E, P], F32R, kind="ExternalInput").ap()
    wkT = nc.dram_tensor("wkT", [E, P], F32R, kind="ExternalInput").ap()
    wvT = nc.dram_tensor("wvT", [E, P], F32R, kind="ExternalInput").ap()
    bqc = nc.dram_tensor("bqc", [P, 1], F32, kind="ExternalInput").ap()
    bkc = nc.dram_tensor("bkc", [P, 1], F32, kind="ExternalInput").ap()
    bvr = nc.dram_tensor("bvr", [P, P], F32, kind="ExternalInput").ap()
    woT = nc.dram_tensor("woT", [E, VS], F32R, kind="ExternalInput").ap()
    bor = nc.dram_tensor("bor", [P, VS], F32, kind="ExternalInput").ap()
    mtri = nc.dram_tensor("mtri", [P, P], F32R, kind="ExternalInput").ap()
    identp = nc.dram_tensor("identp", [P, P], F32R, kind="ExternalInput").ap()
    onesp = nc.dram_tensor("onesp", [P, D], F32R, kind="ExternalInput").ap()

    logits = nc.dram_tensor("logits", [BT, VS], F32, kind="ExternalOutput").ap()
    ce = nc.dram_tensor("ce", [P, BT // P], F32, kind="ExternalOutput").ap()

    TCH = 512                 # t-chunk for phase A
    NA = BT // TCH            # 8
    NT128 = BT // P           # 32 t-chunks of 128
    VCH = 500                 # vocab free chunk (one PSUM bank)
    NV = VS // VCH            # 8
    NS = T // P               # 16 s-chunks per (b,h)
    NQ = T // 512             # 4 oT t-chunks per (b,h)

    with tile.TileContext(nc) as tc:
        with (
            tc.tile_pool(name="consts", bufs=1) as cp,
            tc.tile_pool(name="dram", bufs=1, space="DRAM") as dr,
        ):
            # ---- constants ----
            ident = cp.tile([P, P], F32R)
            nc.sync.dma_start(ident[:], identp[:])
            mtri_sb = cp.tile([P, P], F32R)
            nc.sync.dma_start(mtri_sb[:], mtri[:])
            bq_sb = cp.tile([P, 1], F32)
            nc.sync.dma_start(bq_sb[:], bqc[:])
            bk_sb = cp.tile([P, 1], F32)
            nc.sync.dma_start(bk_sb[:], bkc[:])
            bvr_sb = cp.tile([P, P], F32)
            nc.sync.dma_start(bvr_sb[:], bvr[:])
            ones_sb = cp.tile([P, D], F32R)
            nc.sync.dma_start(ones_sb[:], onesp[:])
            wq_sb = cp.tile([P, E // P, P], F32R)
            nc.sync.dma_start(wq_sb[:], wqT.rearrange("(k p) d -> p k d", p=P))
            wk_sb = cp.tile([P, E // P, P], F32R)
            nc.sync.dma_start(wk_sb[:], wkT.rearrange("(k p) d -> p k d", p=P))
            wv_sb = cp.tile([P, E // P, P], F32R)
            nc.sync.dma_start(wv_sb[:], wvT.rearrange("(k p) d -> p k d", p=P))

            # collective buffers: one AllGather per (batch, 512-col chunk)
            ag_in = [[dr.tile([P, 512], F32R, name=f"agin{b}_{q}")
                      for q in range(NQ)] for b in range(B)]
            ag_out = [[dr.tile([P * NC, 512], F32R, addr_space="Shared",
                               name=f"agout{b}_{q}")
                       for q in range(NQ)] for b in range(B)]

            # Wo first half: pool must outlive qkv (stack order), DMA emitted
            # here but only consumed in phase C -> lands during A/B
            pcW_cm = tc.tile_pool(name="pcW", bufs=1)
            pcW = pcW_cm.__enter__()
            woS1 = pcW.tile([P, E // P, VS // 2], F32R)
            nc.sync.dma_start(
                woS1[:], woT.rearrange("(k p) v -> p k v", p=P)[:, :, :VS // 2])

            qp_cm = tc.tile_pool(name="qkv", bufs=1)
            qp = qp_cm.__enter__()
            # persistent qkv activations (f32r)
            qT = qp.tile([P, BT], F32R)        # [2 heads x 64 d, t]
            kT = qp.tile([P, BT], F32R)
            # v_ext: [t-part, t-chunk, 2 x (64 d + ones col)]
            vx = qp.tile([P, NT128, 2 * (D + 1)], F32R)
            nc.gpsimd.memset(vx[:, :, D:D + 1].bitcast(F32), 1.0)
            nc.gpsimd.memset(vx[:, :, 2 * D + 1:2 * D + 2].bitcast(F32), 1.0)

            # ---------------- Phase A: embed -> xT -> q,k,v ----------------
            with (
                tc.tile_pool(name="pa", bufs=2) as pa,
                tc.tile_pool(name="paT", bufs=2) as paT,
                tc.tile_pool(name="pa_ps", bufs=3, space="PSUM") as pa_ps,
                tc.tile_pool(name="pa_ps2", bufs=3, space="PSUM") as pa_ps2,
            ):
                for a in range(NA):
                    xT = paT.tile([P, E // P, TCH], F32R, tag="xT")
                    for j in range(TCH // P):
                        t128 = a * (TCH // P) + j
                        idx = pa.tile([P, 1], dt.int32, tag="idx")
                        nc.sync.dma_start(idx[:], tokens[t128 * P:(t128 + 1) * P, :])
                        xg = pa.tile([P, E], F32, tag="xg")
                        nc.gpsimd.indirect_dma_start(
                            out=xg[:], out_offset=None, in_=tok_table[:],
                            in_offset=bass.IndirectOffsetOnAxis(ap=idx[:, :1], axis=0),
                        )
                        xp = pa.tile([P, E], F32, tag="xp")
                        tt = (t128 % (T // P)) * P
                        nc.sync.dma_start(xp[:], pos[tt:tt + P, :])
                        xs = pa.tile([P, E], F32R, tag="xs")
                        nc.vector.tensor_tensor(out=xs[:], in0=xg[:], in1=xp[:], op=add_op)
                        for e8 in range(E // P):
                            pst = pa_ps.tile([P, P], F32R, tag="pst")
                            nc.tensor.transpose(pst[:], xs[:, e8 * P:(e8 + 1) * P], ident[:])
                            if e8 % 2 == 0:
                                nc.scalar.copy(xT[:, e8, j * P:(j + 1) * P], pst[:])
                            else:
                                nc.vector.tensor_copy(xT[:, e8, j * P:(j + 1) * P], pst[:])
                    # q,k (transposed layout) + bias on ACT
                    for wsb, b_sb, dst in ((wq_sb, bq_sb, qT), (wk_sb, bk_sb, kT)):
                        psq = pa_ps2.tile([P, TCH], F32, tag="psq")
                        for e8 in range(E // P):
                            nc.tensor.matmul(psq[:], wsb[:, e8, :], xT[:, e8, :],
                                             start=(e8 == 0), stop=(e8 == E // P - 1))
                        nc.scalar.activation(dst[:, a * TCH:(a + 1) * TCH], psq[:],
                                             Ident, bias=b_sb[:])
                    # vT then transpose to v natural (+ bias replicated)
                    psv = pa_ps2.tile([P, TCH], F32, tag="psq")
                    for e8 in range(E // P):
                        nc.tensor.matmul(psv[:], wv_sb[:, e8, :], xT[:, e8, :],
                                         start=(e8 == 0), stop=(e8 == E // P - 1))
                    vTc = pa.tile([P, TCH], F32R, tag="vTc")
                    nc.vector.tensor_copy(out=vTc[:], in_=psv[:])
                    for j in range(TCH // P):
                        t128 = a * (TCH // P) + j
                        psvt = pa_ps.tile([P, P], F32R, tag="pst")
                        nc.tensor.transpose(psvt[:], vTc[:, j * P:(j + 1) * P], ident[:])
                        for h in range(2):
                            nc.vector.tensor_tensor(
                                out=vx[:, t128, h * (D + 1):h * (D + 1) + D],
                                in0=psvt[:, h * D:(h + 1) * D],
                                in1=bvr_sb[:, h * D:(h + 1) * D], op=add_op)

            # ---------------- Phase B: attention ----------------
            with (
                tc.tile_pool(name="pb", bufs=3) as pb,
                tc.tile_pool(name="pb2", bufs=2) as pb2,
                tc.tile_pool(name="pb_ps", bufs=3, space="PSUM") as pb_ps,
                tc.tile_pool(name="pb_psr", bufs=1, space="PSUM") as pb_psr,
                tc.tile_pool(name="pb_ps2", bufs=1, space="PSUM") as pb_ps2,
            ):
                for b in range(B):
                    for h in range(2):
                        qs = qT[h * D:(h + 1) * D, b * T:(b + 1) * T]
                        ks = kT[h * D:(h + 1) * D, b * T:(b + 1) * T]
                        psos = [pb_ps2.tile([D + 1, 512], F32, tag=f"pso{tq}",
                                            name=f"pso{tq}") for tq in range(NQ)]
                        for sb_ in range(0, NS, 2):
                            pair = (sb_, sb_ + 1)
                            pts = {}
                            # scores + exp (interleave the two si's)
                            for tj in range(sb_ // 4, NQ):
                                for si in pair:
                                    if tj < si // 4:
                                        continue
                                    if si not in pts:
                                        pts[si] = pb.tile([P, T], F32R, tag="pt",
                                                          name=f"pt{si}")
                                    psw = pb_ps.tile([P, 512], F32, tag="psw")
                                    nc.tensor.matmul(
                                        psw[:], ks[:, si * P:(si + 1) * P],
                                        qs[:, tj * 512:(tj + 1) * 512],
                                        start=True, stop=True)
                                    nc.scalar.activation(
                                        pts[si][:, tj * 512:(tj + 1) * 512],
                                        psw[:], Exp)
                            # mask the diagonal block / zero invalid prefix
                            for si in pair:
                                m = si % 4
                                toff = (si // 4) * 512
                                if m > 0:
                                    nc.gpsimd.memset(
                                        pts[si][:, toff:toff + P * m].bitcast(F32), 0.0)
                                nc.vector.tensor_tensor(
                                    out=pts[si][:, toff + P * m:toff + P * (m + 1)],
                                    in0=pts[si][:, toff + P * m:toff + P * (m + 1)],
                                    in1=mtri_sb[:], op=mult_op)
                            # oT accumulation (alternating lhsT between the pair)
                            for tq in range(sb_ // 4, NQ):
                                for si in pair:
                                    nc.tensor.matmul(
                                        psos[tq][:],
                                        vx[:, b * NS + si, h * (D + 1):(h + 1) * (D + 1)],
                                        pts[si][:, tq * 512:(tq + 1) * 512],
                                        start=(si == 0), stop=(si == 4 * tq + 3))
                            # tq = sb_//4 completes once pair (4tq+2, 4tq+3) done
                            if sb_ % 4 == 2:
                                tq = sb_ // 4
                                pso = psos[tq]
                                rr = pb2.tile([P, 512], F32R, tag="rr")
                                with nc.allow_low_precision(reason="1/Z feeds f32r bcast"):
                                    nc.vector.reciprocal(rr[D:D + 1, :], pso[D:D + 1, :])
                                psr = pb_psr.tile([D, 512], F32, tag="psr")
                                nc.tensor.matmul(psr[:D], ones_sb[D:D + 1, :],
                                                 rr[D:D + 1, :], start=True, stop=True)
                                rrep = pb2.tile([D, 512], F32R, tag="rrep")
                                nc.scalar.copy(rrep[:], psr[:D])
                                ot = pb2.tile([D, 512], F32R, tag="ot")
                                nc.vector.tensor_tensor(out=ot[:], in0=pso[:D, :],
                                                        in1=rrep[:], op=mult_op)
                                nc.sync.dma_start(
                                    ag_in[b][tq][h * D:(h + 1) * D, :], ot[:])
                                if h == 1:
                                    nc.gpsimd.collective_compute(
                                        "AllGather", mybir.AluOpType.bypass,
                                        replica_groups=[list(range(NC))],
                                        ins=[ag_in[b][tq].opt()],
                                        outs=[ag_out[b][tq].opt()],
                                    )

            qp_cm.__exit__(None, None, None)

            # ================= Phase C: vocab projection + CE =================
            with (
                tc.tile_pool(name="pc", bufs=1) as pc,
                tc.tile_pool(name="pc2", bufs=2) as pc2,
                tc.tile_pool(name="pc3", bufs=3) as pc3,
                tc.tile_pool(name="pc_ps", bufs=1, space="PSUM") as pc_ps,
            ):
                bon = pc.tile([P, VS], F32)
                nc.sync.dma_start(bon[:], bor[:])
                woS2 = pc.tile([P, E // P, VS // 2], F32R)
                nc.sync.dma_start(
                    woS2[:], woT.rearrange("(k p) v -> p k v", p=P)[:, :, VS // 2:])
                ce_acc = pc.tile([P, NT128, NV], F32)
                ce_sb = pc.tile([P, NT128], F32)
                for mI in range(NT128):
                    bI, mL = divmod(mI, T // P)
                    ag_ap = ag_out[bI][mL // 4].opt().rearrange(
                        "(k p) t -> p k t", p=P)
                    otm = pc2.tile([P, E // P, P], F32R, tag="otm")
                    nc.sync.dma_start(
                        otm[:], ag_ap[:, :, (mL % 4) * P:(mL % 4 + 1) * P])
                    psls = [pc_ps.tile([P, VCH], F32, tag=f"psl{n}", name=f"psl{n}")
                            for n in range(NV)]
                    for n in range(NV):
                        woS, off = (woS1, 0) if n < NV // 2 else (woS2, VS // 2)
                        for e8 in range(E // P):
                            nc.tensor.matmul(
                                psls[n][:], otm[:, e8, :],
                                woS[:, e8, n * VCH - off:(n + 1) * VCH - off],
                                start=(e8 == 0), stop=(e8 == E // P - 1))
                    for n in range(NV):
                        lsb = pc3.tile([P, VCH], F32, tag="lsb")
                        nc.vector.tensor_tensor(out=lsb[:], in0=psls[n][:],
                                                in1=bon[:, n * VCH:(n + 1) * VCH],
                                                op=add_op)
                        nc.sync.dma_start(
                            logits[mI * P:(mI + 1) * P, n * VCH:(n + 1) * VCH],
                            lsb[:])
                        esc = pc3.tile([P, VCH], F32, tag="esc")
                        nc.scalar.activation(esc[:], lsb[:], Exp,
                                             accum_out=ce_acc[:, mI, n:n + 1])
                for mI in range(NT128):
                    nc.vector.reduce_sum(out=ce_sb[:, mI:mI + 1], in_=ce_acc[:, mI, :],
                                         axis=mybir.AxisListType.X)
                nc.sync.dma_start(ce[:], ce_sb[:])
            pcW_cm.__exit__(None, None, None)

    nc.compile()
    return nc


def _get_nc():
    if "nc" not in _CACHE:
        _CACHE["nc"] = _build()
    return _CACHE["nc"]


def kernel(tokens, targets, tok_table, pos_emb, Wq, bq, Wk, bk, Wv, bv, Wo, bo):
    from concourse.bass_utils import run_bass_kernel_spmd

    tokens = np.asarray(tokens)
    targets = np.asarray(targets)
    tok_table = np.ascontiguousarray(np.asarray(tok_table, np.float32))
    pos_emb = np.ascontiguousarray(np.asarray(pos_emb, np.float32))
    Wq = np.asarray(Wq, np.float32)
    Wk = np.asarray(Wk, np.float32)
    Wv = np.asarray(Wv, np.float32)
    bq = np.asarray(bq, np.float32)
    bk = np.asarray(bk, np.float32)
    bv = np.asarray(bv, np.float32)
    Wo = np.asarray(Wo, np.float32)
    bo = np.asarray(bo, np.float32)

    tok_i = tokens.reshape(BT, 1).astype(np.int32)
    scale = np.float32(1.0 / np.sqrt(D))  # folded into Wv/bv (quirk: post-softmax)

    # host-side layout prep (pure reshapes/transposes of weights)
    mtri = np.triu(np.ones((P, P), np.float32))  # [s,t] valid iff t >= s
    in_maps = []
    for c in range(NC):
        h0 = HL * c
        wq_c = np.ascontiguousarray(
            Wq[h0:h0 + HL].reshape(HL * D, E).T)            # [E, 128]
        wk_c = np.ascontiguousarray(Wk[h0:h0 + HL].reshape(HL * D, E).T)
        wv_c = np.ascontiguousarray((Wv[h0:h0 + HL] * scale).reshape(HL * D, E).T)
        bq_c = bq[h0:h0 + HL].reshape(P, 1).copy()
        bk_c = bk[h0:h0 + HL].reshape(P, 1).copy()
        bv_c = np.broadcast_to((bv[h0:h0 + HL] * scale).reshape(1, P), (P, P)).copy()
        wo_c = np.ascontiguousarray(Wo[c * VS:(c + 1) * VS].T)  # [E, VS]
        bo_c = np.broadcast_to(bo[c * VS:(c + 1) * VS][None, :], (P, VS)).copy()
        in_maps.append({
            "tokens": tok_i, "tok_table": tok_table, "pos": pos_emb,
            "wqT": wq_c, "wkT": wk_c, "wvT": wv_c,
            "bqc": bq_c, "bkc": bk_c, "bvr": bv_c,
            "woT": wo_c, "bor": bo_c, "mtri": mtri,
            "identp": np.eye(P, dtype=np.float32),
            "onesp": np.ones((P, D), np.float32),
        })

    nc = _get_nc()
    res = run_bass_kernel_spmd(nc, in_maps, core_ids=list(range(NC)), **_RUN_KW)
    global LAST_RESULT
    LAST_RESULT = res

    logits_full = np.empty((BT, V), np.float32)
    S = np.zeros(BT, np.float64)
    for c in range(NC):
        r = res.results[c]
        logits_full[:, c * VS:(c + 1) * VS] = r["logits"]
        S += r["ce"].T.reshape(BT).astype(np.float64)

    tl = targets.reshape(BT).astype(np.int64)
    l_tgt = logits_full[np.arange(BT), tl].astype(np.float64)
    loss = np.float32(np.mean(np.log(S) - l_tgt))
    return logits_full, loss


# revision 31
# speedup vs baseline: 1.1066x; 1.0127x over previous
"""Bigram LM forward on 8 Trainium2 NeuronCores.

Model (B=2, T=2048, E=1024, H=16, D=64, V=32000):
  x = tok_table[tokens] + pos_emb
  q,k,v = per-head projections; causal attention with softmax/sqrt(D)
  (faithful quirk: scale applied AFTER softmax -> fold 1/8 into Wv)
  logits = concat-heads(o) @ Wo.T + bo ; loss = CE(logits, targets)

Sharding: heads across 8 cores (2 each) for attention; vocab (Wo rows)
across 8 cores (4000 each) for the dominant [4096,1024]x[1024,32000]
projection. Eight chunked AllGathers (one per (batch, 512-token
quarter)) move each finished o^T chunk [128,512] -> [1024,512] as soon
as both local heads produced it, so only the last chunk's latency is
exposed. Wo^T streams into SBUF in two halves, the first prefetched
during attention. Per-row CE partials (sum of exp over the vocab
shard) are computed on-device; the host combines partials and
extracts target logits.

All matmuls run in float32r (TF32-like: ~2e-4 rel err, bf16-rate).
Everything is computed "transposed" so no on-chip transpose of the
[T,T] probability matrix is ever needed:
  w^T[s,t] = k^T.T q^T directly (operand swap), softmax normalization
  deferred via a ones-column in v_ext (Z row) + per-row reciprocal
  broadcast with a K=1 matmul. Consecutive matmuls never repeat the
  same stationary operand (measured 9x slowdown when they do).
"""

import numpy as np

B, T, E, H, V = 2, 2048, 1024, 16, 32000
D = E // H
NC = 8
BT = B * T                 # 4096 token rows
HL = H // NC               # 2 heads per core
VS = V // NC               # 4000 vocab rows per core
P = 128

_CACHE = {}
_RUN_KW = {}  # test harness may set e.g. {"trace": True}
LAST_RESULT = None


def _build():
    import concourse.bass as bass
    import concourse.mybir as mybir
    import concourse.tile as tile
    from concourse import bacc

    dt = mybir.dt
    F32 = dt.float32
    F32R = dt.float32r
    Exp = mybir.ActivationFunctionType.Exp
    Ident = mybir.ActivationFunctionType.Identity
    add_op = mybir.AluOpType.add
    mult_op = mybir.AluOpType.mult

    nc = bacc.Bacc("TRN2", target_bir_lowering=False, debug=False,
                   enable_asserts=False, num_devices=NC)

    # ---- DRAM parameters (per-core) ----
    tokens = nc.dram_tensor("tokens", [BT, 1], dt.int32, kind="ExternalInput").ap()
    tok_table = nc.dram_tensor("tok_table", [V, E], F32, kind="ExternalInput").ap()
    pos = nc.dram_tensor("pos", [T, E], F32, kind="ExternalInput").ap()
    wqT = nc.dram_tensor("wqT", [E, P], F32R, kind="ExternalInput").ap()
    wkT = nc.dram_tensor("wkT", [E, P], F32R, kind="ExternalInput").ap()
    wvT = nc.dram_tensor("wvT", [E, P], F32R, kind="ExternalInput").ap()
    bqc = nc.dram_tensor("bqc", [P, 1], F32, kind="ExternalInput").ap()
    bkc = nc.dram_tensor("bkc", [P, 1], F32, kind="ExternalInput").ap()
    bvr = nc.dram_tensor("bvr", [P, P], F32, kind="ExternalInput").ap()
    woT = nc.dram_tensor("woT", [E, VS], F32R, kind="ExternalInput").ap()
    bor = nc.dram_tensor("bor", [P, VS], F32, kind="ExternalInput").ap()
    mtri = nc.dram_tensor("mtri", [P, P], F32R, kind="ExternalInput").ap()
    identp = nc.dram_tensor("identp", [P, P], F32R, kind="ExternalInput").ap()
    onesp = nc.dram_tensor("onesp", [P, D], F32R, kind="ExternalInput").ap()

    logits = nc.dram_tensor("logits", [BT, VS], F32, kind="ExternalOutput").ap()
    ce = nc.dram_tensor("ce", [P, BT // P], F32, kind="ExternalOutput").ap()

    TCH = 512                 # t-chunk for phase A
    NA = BT // TCH            # 8
    NT128 = BT // P           # 32 t-chunks of 128
    VCH = 500                 # vocab free chunk (one PSUM bank)
    NV = VS // VCH            # 8
    NS = T // P               # 16 s-chunks per (b,h)
    NQ = T // 512             # 4 oT t-chunks per (b,h)

    with tile.TileContext(nc) as tc:
        with (
            tc.tile_pool(name="consts", bufs=1) as cp,
            tc.tile_pool(name="dram", bufs=1, space="DRAM") as dr,
        ):
            # ---- constants ----
            ident = cp.tile([P, P], F32R)
            nc.sync.dma_start(ident[:], identp[:])
            mtri_sb = cp.tile([P, P], F32R)
            nc.sync.dma_start(mtri_sb[:], mtri[:])
            bq_sb = cp.tile([P, 1], F32)
            nc.sync.dma_start(bq_sb[:], bqc[:])
            bk_sb = cp.tile([P, 1], F32)
            nc.sync.dma_start(bk_sb[:], bkc[:])
            bvr_sb = cp.tile([P, P], F32)
            nc.sync.dma_start(bvr_sb[:], bvr[:])
            ones_sb = cp.tile([P, D], F32R)
            nc.sync.dma_start(ones_sb[:], onesp[:])
            wq_sb = cp.tile([P, E // P, P], F32R)
            nc.sync.dma_start(wq_sb[:], wqT.rearrange("(k p) d -> p k d", p=P))
            wk_sb = cp.tile([P, E // P, P], F32R)
            nc.sync.dma_start(wk_sb[:], wkT.rearrange("(k p) d -> p k d", p=P))
            wv_sb = cp.tile([P, E // P, P], F32R)
            nc.sync.dma_start(wv_sb[:], wvT.rearrange("(k p) d -> p k d", p=P))

            # collective buffers: one AllGather per (batch, 512-col chunk)
            ag_in = [[dr.tile([P, 512], F32R, name=f"agin{b}_{q}")
                      for q in range(NQ)] for b in range(B)]
            ag_out = [[dr.tile([P * NC, 512], F32R, addr_space="Shared",
                               name=f"agout{b}_{q}")
                       for q in range(NQ)] for b in range(B)]

            # Wo first half: pool must outlive qkv (stack order); its DMA is
            # emitted at the start of phase B (gpsimd queue) so phase A's
            # latency-critical sync DMAs aren't stuck behind 8MB.
            pcW_cm = tc.tile_pool(name="pcW", bufs=1)
            pcW = pcW_cm.__enter__()
            woS1 = pcW.tile([P, E // P, VS // 2], F32R)

            qp_cm = tc.tile_pool(name="qkv", bufs=1)
            qp = qp_cm.__enter__()
            # persistent qkv activations (f32r)
            qT = qp.tile([P, BT], F32R)        # [2 heads x 64 d, t]
            kT = qp.tile([P, BT], F32R)
            # v_ext: [t-part, t-chunk, 2 x (64 d + ones col)]
            vx = qp.tile([P, NT128, 2 * (D + 1)], F32R)
            nc.gpsimd.memset(vx[:, :, D:D + 1].bitcast(F32), 1.0)
            nc.gpsimd.memset(vx[:, :, 2 * D + 1:2 * D + 2].bitcast(F32), 1.0)

            # ---------------- Phase A: embed -> xT -> q,k,v ----------------
            with (
                tc.tile_pool(name="pa", bufs=2) as pa,
                tc.tile_pool(name="paT", bufs=2) as paT,
                tc.tile_pool(name="pa_ps", bufs=3, space="PSUM") as pa_ps,
                tc.tile_pool(name="pa_ps2", bufs=3, space="PSUM") as pa_ps2,
            ):
                for a in range(NA):
                    xT = paT.tile([P, E // P, TCH], F32R, tag="xT")
                    for j in range(TCH // P):
                        t128 = a * (TCH // P) + j
                        idx = pa.tile([P, 1], dt.int32, tag="idx")
                        nc.sync.dma_start(idx[:], tokens[t128 * P:(t128 + 1) * P, :])
                        xg = pa.tile([P, E], F32, tag="xg")
                        nc.gpsimd.indirect_dma_start(
                            out=xg[:], out_offset=None, in_=tok_table[:],
                            in_offset=bass.IndirectOffsetOnAxis(ap=idx[:, :1], axis=0),
                        )
                        xp = pa.tile([P, E], F32, tag="xp")
                        tt = (t128 % (T // P)) * P
                        nc.sync.dma_start(xp[:], pos[tt:tt + P, :])
                        xs = pa.tile([P, E], F32R, tag="xs")
                        nc.vector.tensor_tensor(out=xs[:], in0=xg[:], in1=xp[:], op=add_op)
                        for e8 in range(E // P):
                            pst = pa_ps.tile([P, P], F32R, tag="pst")
                            nc.tensor.transpose(pst[:], xs[:, e8 * P:(e8 + 1) * P], ident[:])
                            if e8 % 2 == 0:
                                nc.scalar.copy(xT[:, e8, j * P:(j + 1) * P], pst[:])
                            else:
                                nc.vector.tensor_copy(xT[:, e8, j * P:(j + 1) * P], pst[:])
                    # q,k (transposed layout) + bias on ACT
                    for wsb, b_sb, dst in ((wq_sb, bq_sb, qT), (wk_sb, bk_sb, kT)):
                        psq = pa_ps2.tile([P, TCH], F32, tag="psq")
                        for e8 in range(E // P):
                            nc.tensor.matmul(psq[:], wsb[:, e8, :], xT[:, e8, :],
                                             start=(e8 == 0), stop=(e8 == E // P - 1))
                        nc.scalar.activation(dst[:, a * TCH:(a + 1) * TCH], psq[:],
                                             Ident, bias=b_sb[:])
                    # vT then transpose to v natural (+ bias replicated)
                    psv = pa_ps2.tile([P, TCH], F32, tag="psq")
                    for e8 in range(E // P):
                        nc.tensor.matmul(psv[:], wv_sb[:, e8, :], xT[:, e8, :],
                                         start=(e8 == 0), stop=(e8 == E // P - 1))
                    vTc = pa.tile([P, TCH], F32R, tag="vTc")
                    nc.vector.tensor_copy(out=vTc[:], in_=psv[:])
                    for j in range(TCH // P):
                        t128 = a * (TCH // P) + j
                        psvt = pa_ps.tile([P, P], F32R, tag="pst")
                        nc.tensor.transpose(psvt[:], vTc[:, j * P:(j + 1) * P], ident[:])
                        for h in range(2):
                            nc.vector.tensor_tensor(
                                out=vx[:, t128, h * (D + 1):h * (D + 1) + D],
                                in0=psvt[:, h * D:(h + 1) * D],
                                in1=bvr_sb[:, h * D:(h + 1) * D], op=add_op)

            # ---------------- Phase B: attention ----------------
            nc.gpsimd.dma_start(
                woS1[:], woT.rearrange("(k p) v -> p k v", p=P)[:, :, :VS // 2])
            with (
                tc.tile_pool(name="pb", bufs=6) as pb,
                tc.tile_pool(name="pb2", bufs=3) as pb2,
                tc.tile_pool(name="pb_ps", bufs=4, space="PSUM") as pb_ps,
                tc.tile_pool(name="pb_ps2", bufs=1, space="PSUM") as pb_ps2,
            ):
                for b in range(B):
                    for h in range(2):
                        qs = qT[h * D:(h + 1) * D, b * T:(b + 1) * T]
                        ks = kT[h * D:(h + 1) * D, b * T:(b + 1) * T]
                        psos = [pb_ps2.tile([D + 1, 512], F32, tag=f"pso{tq}",
                                            name=f"pso{tq}") for tq in range(NQ)]
                        all_pts = {}

                        def scores_pair(sb_):
                            pair = (sb_, sb_ + 1)
                            for tj in range(sb_ // 4, NQ):
                                for si in pair:
                                    if tj < si // 4:
                                        continue
                                    if si not in all_pts:
                                        all_pts[si] = pb.tile([P, T], F32R, tag="pt",
                                                              name=f"pt{si}")
                                    psw = pb_ps.tile([P, 512], F32, tag="psw",
                                                     name="psw")
                                    nc.tensor.matmul(
                                        psw[:], ks[:, si * P:(si + 1) * P],
                                        qs[:, tj * 512:(tj + 1) * 512],
                                        start=True, stop=True)
                                    nc.scalar.activation(
                                        all_pts[si][:, tj * 512:(tj + 1) * 512],
                                        psw[:], Exp)
                            for si in pair:
                                m = si % 4
                                toff = (si // 4) * 512
                                if m > 0:
                                    nc.gpsimd.memset(
                                        all_pts[si][:, toff:toff + P * m].bitcast(F32),
                                        0.0)
                                nc.vector.tensor_tensor(
                                    out=all_pts[si][:, toff + P * m:toff + P * (m + 1)],
                                    in0=all_pts[si][:, toff + P * m:toff + P * (m + 1)],
                                    in1=mtri_sb[:], op=mult_op)

                        def ot_pair(sb_):
                            # oT accumulation (alternating lhsT between the pair)
                            for tq in range(sb_ // 4, NQ):
                                for si in (sb_, sb_ + 1):
                                    nc.tensor.matmul(
                                        psos[tq][:],
                                        vx[:, b * NS + si, h * (D + 1):(h + 1) * (D + 1)],
                                        all_pts[si][:, tq * 512:(tq + 1) * 512],
                                        start=(si == 0), stop=(si == 4 * tq + 3))
                            # tq = sb_//4 completes once pair (4tq+2, 4tq+3) done
                            if sb_ % 4 == 2:
                                tq = sb_ // 4
                                pso = psos[tq]
                                rr = pb2.tile([P, 512], F32R, tag="rr")
                                with nc.allow_low_precision(reason="1/Z feeds f32r bcast"):
                                    nc.vector.reciprocal(rr[D:D + 1, :], pso[D:D + 1, :])
                                psr = pb_ps.tile([D, 512], F32, tag="psw", name="psr")
                                nc.tensor.matmul(psr[:D], ones_sb[D:D + 1, :],
                                                 rr[D:D + 1, :], start=True, stop=True)
                                rrep = pb2.tile([D, 512], F32R, tag="rrep")
                                nc.vector.tensor_copy(rrep[:], psr[:D])
                                ot = pb2.tile([D, 512], F32R, tag="ot")
                                nc.vector.tensor_tensor(out=ot[:], in0=pso[:D, :],
                                                        in1=rrep[:], op=mult_op)
                                nc.sync.dma_start(
                                    ag_in[b][tq][h * D:(h + 1) * D, :], ot[:])
                                if h == 1:
                                    nc.gpsimd.collective_compute(
                                        "AllGather", mybir.AluOpType.bypass,
                                        replica_groups=[list(range(NC))],
                                        ins=[ag_in[b][tq].opt()],
                                        outs=[ag_out[b][tq].opt()],
                                    )

                        # software pipeline: emit oT one pair behind scores so
                        # the PE always has independent matmuls in flight
                        scores_pair(0)
                        for sb_ in range(2, NS, 2):
                            scores_pair(sb_)
                            ot_pair(sb_ - 2)
                        ot_pair(NS - 2)

            qp_cm.__exit__(None, None, None)

            # ================= Phase C: vocab projection + CE =================
            with (
                tc.tile_pool(name="pc", bufs=1) as pc,
                tc.tile_pool(name="pc2", bufs=2) as pc2,
                tc.tile_pool(name="pc3", bufs=3) as pc3,
                tc.tile_pool(name="pc_ps", bufs=1, space="PSUM") as pc_ps,
            ):
                bon = pc.tile([P, VS], F32)
                nc.gpsimd.dma_start(bon[:], bor[:])
                woS2 = pc.tile([P, E // P, VS // 2], F32R)
                nc.gpsimd.dma_start(
                    woS2[:], woT.rearrange("(k p) v -> p k v", p=P)[:, :, VS // 2:])
                ce_acc = pc.tile([P, NT128, NV], F32)
                ce_sb = pc.tile([P, NT128], F32)
                for mI in range(NT128):
                    bI, mL = divmod(mI, T // P)
                    ag_ap = ag_out[bI][mL // 4].opt().rearrange(
                        "(k p) t -> p k t", p=P)
                    otm = pc2.tile([P, E // P, P], F32R, tag="otm")
                    nc.sync.dma_start(
                        otm[:], ag_ap[:, :, (mL % 4) * P:(mL % 4 + 1) * P])
                    psls = [pc_ps.tile([P, VCH], F32, tag=f"psl{n}", name=f"psl{n}")
                            for n in range(NV)]
                    for n in range(NV):
                        woS, off = (woS1, 0) if n < NV // 2 else (woS2, VS // 2)
                        for e8 in range(E // P):
                            nc.tensor.matmul(
                                psls[n][:], otm[:, e8, :],
                                woS[:, e8, n * VCH - off:(n + 1) * VCH - off],
                                start=(e8 == 0), stop=(e8 == E // P - 1))
                    for n in range(NV):
                        lsb = pc3.tile([P, VCH], F32, tag="lsb")
                        nc.vector.tensor_tensor(out=lsb[:], in0=psls[n][:],
                                                in1=bon[:, n * VCH:(n + 1) * VCH],
                                                op=add_op)
                        nc.sync.dma_start(
                            logits[mI * P:(mI + 1) * P, n * VCH:(n + 1) * VCH],
                            lsb[:])
                        esc = pc3.tile([P, VCH], F32, tag="esc")
                        nc.scalar.activation(esc[:], lsb[:], Exp,
                                             accum_out=ce_acc[:, mI, n:n + 1])
                for mI in range(NT128):
                    nc.vector.reduce_sum(out=ce_sb[:, mI:mI + 1], in_=ce_acc[:, mI, :],
                                         axis=mybir.AxisListType.X)
                nc.sync.dma_start(ce[:], ce_sb[:])
            pcW_cm.__exit__(None, None, None)

    nc.compile()
    return nc


def _get_nc():
    if "nc" not in _CACHE:
        _CACHE["nc"] = _build()
    return _CACHE["nc"]


def kernel(tokens, targets, tok_table, pos_emb, Wq, bq, Wk, bk, Wv, bv, Wo, bo):
    from concourse.bass_utils import run_bass_kernel_spmd

    tokens = np.asarray(tokens)
    targets = np.asarray(targets)
    tok_table = np.ascontiguousarray(np.asarray(tok_table, np.float32))
    pos_emb = np.ascontiguousarray(np.asarray(pos_emb, np.float32))
    Wq = np.asarray(Wq, np.float32)
    Wk = np.asarray(Wk, np.float32)
    Wv = np.asarray(Wv, np.float32)
    bq = np.asarray(bq, np.float32)
    bk = np.asarray(bk, np.float32)
    bv = np.asarray(bv, np.float32)
    Wo = np.asarray(Wo, np.float32)
    bo = np.asarray(bo, np.float32)

    tok_i = tokens.reshape(BT, 1).astype(np.int32)
    scale = np.float32(1.0 / np.sqrt(D))  # folded into Wv/bv (quirk: post-softmax)

    # host-side layout prep (pure reshapes/transposes of weights)
    mtri = np.triu(np.ones((P, P), np.float32))  # [s,t] valid iff t >= s
    in_maps = []
    for c in range(NC):
        h0 = HL * c
        wq_c = np.ascontiguousarray(
            Wq[h0:h0 + HL].reshape(HL * D, E).T)            # [E, 128]
        wk_c = np.ascontiguousarray(Wk[h0:h0 + HL].reshape(HL * D, E).T)
        wv_c = np.ascontiguousarray((Wv[h0:h0 + HL] * scale).reshape(HL * D, E).T)
        bq_c = bq[h0:h0 + HL].reshape(P, 1).copy()
        bk_c = bk[h0:h0 + HL].reshape(P, 1).copy()
        bv_c = np.broadcast_to((bv[h0:h0 + HL] * scale).reshape(1, P), (P, P)).copy()
        wo_c = np.ascontiguousarray(Wo[c * VS:(c + 1) * VS].T)  # [E, VS]
        bo_c = np.broadcast_to(bo[c * VS:(c + 1) * VS][None, :], (P, VS)).copy()
        in_maps.append({
            "tokens": tok_i, "tok_table": tok_table, "pos": pos_emb,
            "wqT": wq_c, "wkT": wk_c, "wvT": wv_c,
            "bqc": bq_c, "bkc": bk_c, "bvr": bv_c,
            "woT": wo_c, "bor": bo_c, "mtri": mtri,
            "identp": np.eye(P, dtype=np.float32),
            "onesp": np.ones((P, D), np.float32),
        })

    nc = _get_nc()
    res = run_bass_kernel_spmd(nc, in_maps, core_ids=list(range(NC)), **_RUN_KW)
    global LAST_RESULT
    LAST_RESULT = res

    logits_full = np.empty((BT, V), np.float32)
    S = np.zeros(BT, np.float64)
    for c in range(NC):
        r = res.results[c]
        logits_full[:, c * VS:(c + 1) * VS] = r["logits"]
        S += r["ce"].T.reshape(BT).astype(np.float64)

    tl = targets.reshape(BT).astype(np.int64)
    l_tgt = logits_full[np.arange(BT), tl].astype(np.float64)
    loss = np.float32(np.mean(np.log(S) - l_tgt))
    return logits_full, loss


# revision 32
# speedup vs baseline: 1.1287x; 1.0200x over previous
"""Bigram LM forward on 8 Trainium2 NeuronCores.

Model (B=2, T=2048, E=1024, H=16, D=64, V=32000):
  x = tok_table[tokens] + pos_emb
  q,k,v = per-head projections; causal attention with softmax/sqrt(D)
  (faithful quirk: scale applied AFTER softmax -> fold 1/8 into Wv)
  logits = concat-heads(o) @ Wo.T + bo ; loss = CE(logits, targets)

Sharding: heads across 8 cores (2 each) for attention; vocab (Wo rows)
across 8 cores (4000 each) for the dominant [4096,1024]x[1024,32000]
projection. Eight chunked AllGathers (one per (batch, 512-token
quarter)) move each finished o^T chunk [128,512] -> [1024,512] as soon
as both local heads produced it, so only the last chunk's latency is
exposed. Wo^T streams into SBUF in two halves; phase C runs two vocab
half-sweeps so compute starts on the prefetched first half while the
second loads. Per-row CE partials (sum of exp over the vocab shard)
are computed on-device; the host combines partials and extracts
target logits.

Matmuls run in float32r (TF32-like: ~2e-4 rel err, bf16 rate); q/k are
bf16 (score errors vanish through softmax). Everything is computed
"transposed" so no on-chip transpose of the [T,T] probability matrix
is ever needed: w^T[s,t] = k^T.T q^T directly (operand swap), softmax
normalization deferred via a ones-column in v_ext (Z row) + per-row
reciprocal broadcast with a K=1 matmul.
"""

import numpy as np

B, T, E, H, V = 2, 2048, 1024, 16, 32000
D = E // H
NC = 8
BT = B * T                 # 4096 token rows
HL = H // NC               # 2 heads per core
VS = V // NC               # 4000 vocab rows per core
P = 128

_CACHE = {}
_RUN_KW = {}  # test harness may set e.g. {"trace": True}
LAST_RESULT = None


def _build():
    import concourse.bass as bass
    import concourse.mybir as mybir
    import concourse.tile as tile
    from concourse import bacc

    dt = mybir.dt
    F32 = dt.float32
    F32R = dt.float32r
    BF16 = dt.bfloat16
    Exp = mybir.ActivationFunctionType.Exp
    Ident = mybir.ActivationFunctionType.Identity
    add_op = mybir.AluOpType.add
    mult_op = mybir.AluOpType.mult

    nc = bacc.Bacc("TRN2", target_bir_lowering=False, debug=False,
                   enable_asserts=False, num_devices=NC)

    # ---- DRAM parameters (per-core); weight layouts prepped on host ----
    tokens = nc.dram_tensor("tokens", [BT, 1], dt.int32, kind="ExternalInput").ap()
    tok_table = nc.dram_tensor("tok_table", [V, E], F32, kind="ExternalInput").ap()
    pos = nc.dram_tensor("pos", [T, E], F32, kind="ExternalInput").ap()
    wqT = nc.dram_tensor("wqT", [P, E // P, P], F32R, kind="ExternalInput").ap()
    wkT = nc.dram_tensor("wkT", [P, E // P, P], F32R, kind="ExternalInput").ap()
    wvT = nc.dram_tensor("wvT", [P, E // P, P], F32R, kind="ExternalInput").ap()
    bqc = nc.dram_tensor("bqc", [P, 1], F32, kind="ExternalInput").ap()
    bkc = nc.dram_tensor("bkc", [P, 1], F32, kind="ExternalInput").ap()
    bvr = nc.dram_tensor("bvr", [P, P], F32, kind="ExternalInput").ap()
    woTp = nc.dram_tensor("woTp", [P, E // P, VS], F32R, kind="ExternalInput").ap()
    bor = nc.dram_tensor("bor", [P, VS], F32, kind="ExternalInput").ap()
    mask4 = nc.dram_tensor("mask4", [P, 4, 512], F32R, kind="ExternalInput").ap()
    identp = nc.dram_tensor("identp", [P, P], F32R, kind="ExternalInput").ap()
    onesp = nc.dram_tensor("onesp", [P, D], F32R, kind="ExternalInput").ap()

    logits = nc.dram_tensor("logits", [BT, VS], F32, kind="ExternalOutput").ap()
    ce = nc.dram_tensor("ce", [P, BT // P], F32, kind="ExternalOutput").ap()

    TCH = 512                 # t-chunk for phase A
    NA = BT // TCH            # 8
    NT128 = BT // P           # 32 t-chunks of 128
    VCH = 500                 # vocab free chunk (one PSUM bank)
    NV = VS // VCH            # 8
    NS = T // P               # 16 s-chunks per (b,h)
    NQ = T // 512             # 4 oT t-chunks per (b,h)

    with tile.TileContext(nc) as tc:
        with (
            tc.tile_pool(name="consts", bufs=1) as cp,
            tc.tile_pool(name="dram", bufs=1, space="DRAM") as dr,
        ):
            # ---- constants (all contiguous loads) ----
            ident = cp.tile([P, P], F32R)
            nc.sync.dma_start(ident[:], identp[:])
            mask_sb = cp.tile([P, 4, 512], F32R)
            nc.sync.dma_start(mask_sb[:], mask4[:])
            bq_sb = cp.tile([P, 1], F32)
            nc.sync.dma_start(bq_sb[:], bqc[:])
            bk_sb = cp.tile([P, 1], F32)
            nc.sync.dma_start(bk_sb[:], bkc[:])
            bvr_sb = cp.tile([P, P], F32)
            nc.sync.dma_start(bvr_sb[:], bvr[:])
            ones_sb = cp.tile([P, D], F32R)
            nc.sync.dma_start(ones_sb[:], onesp[:])
            wq_sb = cp.tile([P, E // P, P], F32R)
            nc.sync.dma_start(wq_sb[:], wqT[:])
            wk_sb = cp.tile([P, E // P, P], F32R)
            nc.sync.dma_start(wk_sb[:], wkT[:])
            wv_sb = cp.tile([P, E // P, P], F32R)
            nc.sync.dma_start(wv_sb[:], wvT[:])

            # collective buffers: one AllGather per (batch, 512-col chunk)
            ag_in = [[dr.tile([P, 512], F32R, name=f"agin{b}_{q}")
                      for q in range(NQ)] for b in range(B)]
            ag_out = [[dr.tile([P * NC, 512], F32R, addr_space="Shared",
                               name=f"agout{b}_{q}")
                       for q in range(NQ)] for b in range(B)]

            # Wo first half: pool outlives qkv (stack order); DMA emitted at
            # phase B start on the gpsimd queue.
            pcW_cm = tc.tile_pool(name="pcW", bufs=1)
            pcW = pcW_cm.__enter__()
            woS1 = pcW.tile([P, E // P, VS // 2], F32R)

            qp_cm = tc.tile_pool(name="qkv", bufs=1)
            qp = qp_cm.__enter__()
            # persistent qkv activations
            qT = qp.tile([P, BT], BF16)        # [2 heads x 64 d, t]
            kT = qp.tile([P, BT], BF16)
            # v_ext: [t-part, t-chunk, 2 x (64 d + ones col)]  (f32r)
            vx = qp.tile([P, NT128, 2 * (D + 1)], F32R)
            nc.gpsimd.memset(vx[:, :, D:D + 1].bitcast(F32), 1.0)
            nc.gpsimd.memset(vx[:, :, 2 * D + 1:2 * D + 2].bitcast(F32), 1.0)

            # ---------------- Phase A: embed -> xT -> q,k,v ----------------
            with (
                tc.tile_pool(name="pa", bufs=3) as pa,
                tc.tile_pool(name="paT", bufs=2) as paT,
                tc.tile_pool(name="paV", bufs=1) as paV,
                tc.tile_pool(name="pa_ps", bufs=3, space="PSUM") as pa_ps,
                tc.tile_pool(name="pa_ps2", bufs=3, space="PSUM") as pa_ps2,
            ):
                for a in range(NA):
                    xT = paT.tile([P, E // P, TCH], F32R, tag="xT")
                    for j in range(TCH // P):
                        t128 = a * (TCH // P) + j
                        idx = pa.tile([P, 1], dt.int32, tag="idx")
                        nc.sync.dma_start(idx[:], tokens[t128 * P:(t128 + 1) * P, :])
                        xg = pa.tile([P, E], F32, tag="xg")
                        nc.gpsimd.indirect_dma_start(
                            out=xg[:], out_offset=None, in_=tok_table[:],
                            in_offset=bass.IndirectOffsetOnAxis(ap=idx[:, :1], axis=0),
                        )
                        xp = pa.tile([P, E], F32, tag="xp")
                        tt = (t128 % (T // P)) * P
                        eng = nc.sync if j % 2 == 0 else nc.gpsimd
                        eng.dma_start(xp[:], pos[tt:tt + P, :])
                        xs = pa.tile([P, E], F32R, tag="xs")
                        nc.vector.tensor_tensor(out=xs[:], in0=xg[:], in1=xp[:], op=add_op)
                        for e8 in range(E // P):
                            pst = pa_ps.tile([P, P], F32R, tag="pst")
                            nc.tensor.transpose(pst[:], xs[:, e8 * P:(e8 + 1) * P], ident[:])
                            if e8 % 2 == 0:
                                nc.scalar.copy(xT[:, e8, j * P:(j + 1) * P], pst[:])
                            else:
                                nc.vector.tensor_copy(xT[:, e8, j * P:(j + 1) * P], pst[:])
                    # q,k (transposed layout, bf16) + bias on ACT
                    for wsb, b_sb, dst in ((wq_sb, bq_sb, qT), (wk_sb, bk_sb, kT)):
                        psq = pa_ps2.tile([P, TCH], F32, tag="psq")
                        for e8 in range(E // P):
                            nc.tensor.matmul(psq[:], wsb[:, e8, :], xT[:, e8, :],
                                             start=(e8 == 0), stop=(e8 == E // P - 1))
                        nc.scalar.activation(dst[:, a * TCH:(a + 1) * TCH], psq[:],
                                             Ident, bias=b_sb[:])
                    # vT then transpose to v natural (+ bias replicated)
                    psv = pa_ps2.tile([P, TCH], F32, tag="psq")
                    for e8 in range(E // P):
                        nc.tensor.matmul(psv[:], wv_sb[:, e8, :], xT[:, e8, :],
                                         start=(e8 == 0), stop=(e8 == E // P - 1))
                    vTc = paV.tile([P, TCH], F32R, tag="vTc")
                    nc.vector.tensor_copy(out=vTc[:], in_=psv[:])
                    for j in range(TCH // P):
                        t128 = a * (TCH // P) + j
                        psvt = pa_ps.tile([P, P], F32R, tag="pst")
                        nc.tensor.transpose(psvt[:], vTc[:, j * P:(j + 1) * P], ident[:])
                        for h in range(2):
                            nc.vector.tensor_tensor(
                                out=vx[:, t128, h * (D + 1):h * (D + 1) + D],
                                in0=psvt[:, h * D:(h + 1) * D],
                                in1=bvr_sb[:, h * D:(h + 1) * D], op=add_op)

            # ---------------- Phase B: attention ----------------
            nc.gpsimd.dma_start(woS1[:], woTp[:, :, :VS // 2])
            with (
                tc.tile_pool(name="pb", bufs=6) as pb,
                tc.tile_pool(name="pb2", bufs=3) as pb2,
                tc.tile_pool(name="pb_ps", bufs=4, space="PSUM") as pb_ps,
                tc.tile_pool(name="pb_ps2", bufs=1, space="PSUM") as pb_ps2,
            ):
                for b in range(B):
                    for h in range(2):
                        qs = qT[h * D:(h + 1) * D, b * T:(b + 1) * T]
                        ks = kT[h * D:(h + 1) * D, b * T:(b + 1) * T]
                        psos = [pb_ps2.tile([D + 1, 512], F32, tag=f"pso{tq}",
                                            name=f"pso{tq}") for tq in range(NQ)]
                        all_pts = {}

                        def scores_pair(sb_):
                            pair = (sb_, sb_ + 1)
                            for tj in range(sb_ // 4, NQ):
                                for si in pair:
                                    if tj < si // 4:
                                        continue
                                    if si not in all_pts:
                                        all_pts[si] = pb.tile([P, T], F32R, tag="pt",
                                                              name=f"pt{si}")
                                    psw = pb_ps.tile([P, 512], F32, tag="psw",
                                                     name="psw")
                                    nc.tensor.matmul(
                                        psw[:], ks[:, si * P:(si + 1) * P],
                                        qs[:, tj * 512:(tj + 1) * 512],
                                        start=True, stop=True)
                                    nc.scalar.activation(
                                        all_pts[si][:, tj * 512:(tj + 1) * 512],
                                        psw[:], Exp)
                            for si in pair:
                                m = si % 4
                                toff = (si // 4) * 512
                                w = P * (m + 1)
                                nc.vector.tensor_tensor(
                                    out=all_pts[si][:, toff:toff + w],
                                    in0=all_pts[si][:, toff:toff + w],
                                    in1=mask_sb[:, m, :w], op=mult_op)

                        def ot_pair(sb_):
                            # oT accumulation (alternating lhsT between the pair)
                            for tq in range(sb_ // 4, NQ):
                                for si in (sb_, sb_ + 1):
                                    nc.tensor.matmul(
                                        psos[tq][:],
                                        vx[:, b * NS + si, h * (D + 1):(h + 1) * (D + 1)],
                                        all_pts[si][:, tq * 512:(tq + 1) * 512],
                                        start=(si == 0), stop=(si == 4 * tq + 3))
                            # tq = sb_//4 completes once pair (4tq+2, 4tq+3) done
                            if sb_ % 4 == 2:
                                tq = sb_ // 4
                                pso = psos[tq]
                                rr = pb2.tile([P, 512], F32R, tag="rr")
                                with nc.allow_low_precision(reason="1/Z feeds f32r bcast"):
                                    nc.vector.reciprocal(rr[D:D + 1, :], pso[D:D + 1, :])
                                psr = pb_ps.tile([D, 512], F32, tag="psw", name="psr")
                                nc.tensor.matmul(psr[:D], ones_sb[D:D + 1, :],
                                                 rr[D:D + 1, :], start=True, stop=True)
                                rrep = pb2.tile([D, 512], F32R, tag="rrep")
                                nc.vector.tensor_copy(rrep[:], psr[:D])
                                ot = pb2.tile([D, 512], F32R, tag="ot")
                                nc.vector.tensor_tensor(out=ot[:], in0=pso[:D, :],
                                                        in1=rrep[:], op=mult_op)
                                nc.sync.dma_start(
                                    ag_in[b][tq][h * D:(h + 1) * D, :], ot[:])
                                if h == 1:
                                    nc.gpsimd.collective_compute(
                                        "AllGather", mybir.AluOpType.bypass,
                                        replica_groups=[list(range(NC))],
                                        ins=[ag_in[b][tq].opt()],
                                        outs=[ag_out[b][tq].opt()],
                                    )

                        # software pipeline depth 2: scores two pairs ahead of oT
                        scores_pair(0)
                        scores_pair(2)
                        for sb_ in range(4, NS, 2):
                            scores_pair(sb_)
                            ot_pair(sb_ - 4)
                        ot_pair(NS - 4)
                        ot_pair(NS - 2)

            qp_cm.__exit__(None, None, None)

            # ================= Phase C: vocab projection + CE =================
            with (
                tc.tile_pool(name="pc", bufs=1) as pc,
                tc.tile_pool(name="pc2", bufs=2) as pc2,
                tc.tile_pool(name="pc3", bufs=3) as pc3,
                tc.tile_pool(name="pc_ps", bufs=1, space="PSUM") as pc_ps,
            ):
                bon = pc.tile([P, VS], F32)
                nc.gpsimd.dma_start(bon[:], bor[:])
                woS2 = pc.tile([P, E // P, VS // 2], F32R)
                nc.gpsimd.dma_start(woS2[:], woTp[:, :, VS // 2:])
                ce_acc = pc.tile([P, NT128, NV], F32)
                ce_sb = pc.tile([P, NT128], F32)
                for half in range(2):
                    woS, off = (woS1, 0) if half == 0 else (woS2, VS // 2)
                    for mI in range(NT128):
                        bI, mL = divmod(mI, T // P)
                        ag_ap = ag_out[bI][mL // 4].opt().rearrange(
                            "(k p) t -> p k t", p=P)
                        otm = pc2.tile([P, E // P, P], F32R, tag="otm")
                        nc.sync.dma_start(
                            otm[:], ag_ap[:, :, (mL % 4) * P:(mL % 4 + 1) * P])
                        psls = [pc_ps.tile([P, VCH], F32, tag=f"psl{n}",
                                           name=f"psl{n}")
                                for n in range(NV // 2)]
                        for nn in range(NV // 2):
                            n = half * (NV // 2) + nn
                            for e8 in range(E // P):
                                nc.tensor.matmul(
                                    psls[nn][:], otm[:, e8, :],
                                    woS[:, e8, n * VCH - off:(n + 1) * VCH - off],
                                    start=(e8 == 0), stop=(e8 == E // P - 1))
                        for nn in range(NV // 2):
                            n = half * (NV // 2) + nn
                            lsb = pc3.tile([P, VCH], F32, tag="lsb")
                            nc.vector.tensor_tensor(out=lsb[:], in0=psls[nn][:],
                                                    in1=bon[:, n * VCH:(n + 1) * VCH],
                                                    op=add_op)
                            nc.sync.dma_start(
                                logits[mI * P:(mI + 1) * P, n * VCH:(n + 1) * VCH],
                                lsb[:])
                            esc = pc3.tile([P, VCH], F32, tag="esc")
                            nc.scalar.activation(esc[:], lsb[:], Exp,
                                                 accum_out=ce_acc[:, mI, n:n + 1])
                for mI in range(NT128):
                    nc.vector.reduce_sum(out=ce_sb[:, mI:mI + 1], in_=ce_acc[:, mI, :],
                                         axis=mybir.AxisListType.X)
                nc.sync.dma_start(ce[:], ce_sb[:])
            pcW_cm.__exit__(None, None, None)

    nc.compile()
    return nc


def _get_nc():
    if "nc" not in _CACHE:
        _CACHE["nc"] = _build()
    return _CACHE["nc"]


def kernel(tokens, targets, tok_table, pos_emb, Wq, bq, Wk, bk, Wv, bv, Wo, bo):
    from concourse.bass_utils import run_bass_kernel_spmd

    tokens = np.asarray(tokens)
    targets = np.asarray(targets)
    tok_table = np.ascontiguousarray(np.asarray(tok_table, np.float32))
    pos_emb = np.ascontiguousarray(np.asarray(pos_emb, np.float32))
    Wq = np.asarray(Wq, np.float32)
    Wk = np.asarray(Wk, np.float32)
    Wv = np.asarray(Wv, np.float32)
    bq = np.asarray(bq, np.float32)
    bk = np.asarray(bk, np.float32)
    bv = np.asarray(bv, np.float32)
    Wo = np.asarray(Wo, np.float32)
    bo = np.asarray(bo, np.float32)

    tok_i = tokens.reshape(BT, 1).astype(np.int32)
    scale = np.float32(1.0 / np.sqrt(D))  # folded into Wv/bv (quirk: post-softmax)

    # host-side layout prep (pure reshapes/transposes of weights)
    # mask4[m][sp, c] = 1 iff c >= 128*m + sp   (prefix zeros + shifted triu)
    cidx = np.arange(512)[None, :]
    spidx = np.arange(P)[:, None]
    mask4 = np.stack([(cidx >= 128 * m + spidx).astype(np.float32)
                      for m in range(4)], axis=1)         # [128, 4, 512]
    mask4 = np.ascontiguousarray(mask4)

    def pk(w):  # [E, X] -> [128, 8, X] contiguous (partition-major K-subtiles)
        return np.ascontiguousarray(w.reshape(E // P, P, -1).transpose(1, 0, 2))

    in_maps = []
    for c in range(NC):
        h0 = HL * c
        wq_c = pk(Wq[h0:h0 + HL].reshape(HL * D, E).T)
        wk_c = pk(Wk[h0:h0 + HL].reshape(HL * D, E).T)
        wv_c = pk((Wv[h0:h0 + HL] * scale).reshape(HL * D, E).T)
        bq_c = bq[h0:h0 + HL].reshape(P, 1).copy()
        bk_c = bk[h0:h0 + HL].reshape(P, 1).copy()
        bv_c = np.broadcast_to((bv[h0:h0 + HL] * scale).reshape(1, P), (P, P)).copy()
        wo_c = pk(Wo[c * VS:(c + 1) * VS].T)              # [128, 8, VS]
        bo_c = np.broadcast_to(bo[c * VS:(c + 1) * VS][None, :], (P, VS)).copy()
        in_maps.append({
            "tokens": tok_i, "tok_table": tok_table, "pos": pos_emb,
            "wqT": wq_c, "wkT": wk_c, "wvT": wv_c,
            "bqc": bq_c, "bkc": bk_c, "bvr": bv_c,
            "woTp": wo_c, "bor": bo_c, "mask4": mask4,
            "identp": np.eye(P, dtype=np.float32),
            "onesp": np.ones((P, D), np.float32),
        })

    nc = _get_nc()
    res = run_bass_kernel_spmd(nc, in_maps, core_ids=list(range(NC)), **_RUN_KW)
    global LAST_RESULT
    LAST_RESULT = res

    logits_full = np.empty((BT, V), np.float32)
    S = np.zeros(BT, np.float64)
    for c in range(NC):
        r = res.results[c]
        logits_full[:, c * VS:(c + 1) * VS] = r["logits"]
        S += r["ce"].T.reshape(BT).astype(np.float64)

    tl = targets.reshape(BT).astype(np.int64)
    l_tgt = logits_full[np.arange(BT), tl].astype(np.float64)
    loss = np.float32(np.mean(np.log(S) - l_tgt))
    return logits_full, loss


# revision 34
# speedup vs baseline: 1.1363x; 1.0067x over previous
"""Bigram LM forward on 8 Trainium2 NeuronCores.

Model (B=2, T=2048, E=1024, H=16, D=64, V=32000):
  x = tok_table[tokens] + pos_emb
  q,k,v = per-head projections; causal attention with softmax/sqrt(D)
  (faithful quirk: scale applied AFTER softmax -> fold 1/8 into Wv)
  logits = concat-heads(o) @ Wo.T + bo ; loss = CE(logits, targets)

Sharding: heads across 8 cores (2 each) for attention; vocab (Wo rows)
across 8 cores (4000 each) for the dominant [4096,1024]x[1024,32000]
projection. Eight chunked AllGathers (one per (batch, 512-token
quarter)) move each finished o^T chunk [128,512] -> [1024,512] as soon
as both local heads produced it, so only the last chunk's latency is
exposed. Wo^T streams into SBUF in two halves; phase C runs two vocab
half-sweeps so compute starts on the prefetched first half while the
second loads. Per-row CE partials (sum of exp over the vocab shard)
are computed on-device; the host combines partials and extracts
target logits.

Matmuls run in float32r (TF32-like: ~2e-4 rel err, bf16 rate); q/k are
bf16 (score errors vanish through softmax). Everything is computed
"transposed" so no on-chip transpose of the [T,T] probability matrix
is ever needed: w^T[s,t] = k^T.T q^T directly (operand swap), softmax
normalization deferred via a ones-column in v_ext (Z row) + per-row
reciprocal broadcast with a K=1 matmul.
"""

import numpy as np

B, T, E, H, V = 2, 2048, 1024, 16, 32000
D = E // H
NC = 8
BT = B * T                 # 4096 token rows
HL = H // NC               # 2 heads per core
VS = V // NC               # 4000 vocab rows per core
P = 128

_CACHE = {}
_RUN_KW = {}  # test harness may set e.g. {"trace": True}
LAST_RESULT = None


def _build():
    import concourse.bass as bass
    import concourse.mybir as mybir
    import concourse.tile as tile
    from concourse import bacc

    dt = mybir.dt
    F32 = dt.float32
    F32R = dt.float32r
    BF16 = dt.bfloat16
    Exp = mybir.ActivationFunctionType.Exp
    Ident = mybir.ActivationFunctionType.Identity
    add_op = mybir.AluOpType.add
    mult_op = mybir.AluOpType.mult

    nc = bacc.Bacc("TRN2", target_bir_lowering=False, debug=False,
                   enable_asserts=False, num_devices=NC)

    # ---- DRAM parameters (per-core); weight layouts prepped on host ----
    tokens = nc.dram_tensor("tokens", [BT, 1], dt.int32, kind="ExternalInput").ap()
    tok_table = nc.dram_tensor("tok_table", [V, E], F32, kind="ExternalInput").ap()
    pos = nc.dram_tensor("pos", [T, E], F32, kind="ExternalInput").ap()
    wqT = nc.dram_tensor("wqT", [P, E // P, P], F32R, kind="ExternalInput").ap()
    wkT = nc.dram_tensor("wkT", [P, E // P, P], F32R, kind="ExternalInput").ap()
    wvT = nc.dram_tensor("wvT", [P, E // P, P], F32R, kind="ExternalInput").ap()
    bqc = nc.dram_tensor("bqc", [P, 1], F32, kind="ExternalInput").ap()
    bkc = nc.dram_tensor("bkc", [P, 1], F32, kind="ExternalInput").ap()
    bvr = nc.dram_tensor("bvr", [P, P], F32, kind="ExternalInput").ap()
    woTp = nc.dram_tensor("woTp", [P, E // P, VS], F32R, kind="ExternalInput").ap()
    bor = nc.dram_tensor("bor", [P, VS], F32, kind="ExternalInput").ap()
    mask4 = nc.dram_tensor("mask4", [P, 4, 512], F32R, kind="ExternalInput").ap()
    identp = nc.dram_tensor("identp", [P, P], F32R, kind="ExternalInput").ap()
    onesp = nc.dram_tensor("onesp", [P, D], F32R, kind="ExternalInput").ap()

    logits = nc.dram_tensor("logits", [BT, VS], F32, kind="ExternalOutput").ap()
    ce = nc.dram_tensor("ce", [P, BT // P], F32, kind="ExternalOutput").ap()

    TCH = 512                 # t-chunk for phase A
    NA = BT // TCH            # 8
    NT128 = BT // P           # 32 t-chunks of 128
    VCH = 500                 # vocab free chunk (one PSUM bank)
    NV = VS // VCH            # 8
    NS = T // P               # 16 s-chunks per (b,h)
    NQ = T // 512             # 4 oT t-chunks per (b,h)

    with tile.TileContext(nc) as tc:
        with (
            tc.tile_pool(name="consts", bufs=1) as cp,
            tc.tile_pool(name="dram", bufs=1, space="DRAM") as dr,
        ):
            # ---- constants (all contiguous loads) ----
            ident = cp.tile([P, P], F32R)
            nc.sync.dma_start(ident[:], identp[:])
            mask_sb = cp.tile([P, 4, 512], F32R)
            nc.sync.dma_start(mask_sb[:], mask4[:])
            bq_sb = cp.tile([P, 1], F32)
            nc.sync.dma_start(bq_sb[:], bqc[:])
            bk_sb = cp.tile([P, 1], F32)
            nc.sync.dma_start(bk_sb[:], bkc[:])
            bvr_sb = cp.tile([P, P], F32)
            nc.sync.dma_start(bvr_sb[:], bvr[:])
            ones_sb = cp.tile([P, D], F32R)
            nc.sync.dma_start(ones_sb[:], onesp[:])
            wq_sb = cp.tile([P, E // P, P], F32R)
            nc.sync.dma_start(wq_sb[:], wqT[:])
            wk_sb = cp.tile([P, E // P, P], F32R)
            nc.sync.dma_start(wk_sb[:], wkT[:])
            wv_sb = cp.tile([P, E // P, P], F32R)
            nc.sync.dma_start(wv_sb[:], wvT[:])

            # collective buffers: one AllGather per (batch, 512-col chunk)
            ag_in = [[dr.tile([P, 512], F32R, name=f"agin{b}_{q}")
                      for q in range(NQ)] for b in range(B)]
            ag_out = [[dr.tile([P * NC, 512], F32R, addr_space="Shared",
                               name=f"agout{b}_{q}")
                       for q in range(NQ)] for b in range(B)]

            # Wo first half: pool outlives qkv (stack order); DMA emitted at
            # phase B start on the gpsimd queue.
            pcW_cm = tc.tile_pool(name="pcW", bufs=1)
            pcW = pcW_cm.__enter__()
            woS1 = pcW.tile([P, E // P, VS // 2], F32R)

            qp_cm = tc.tile_pool(name="qkv", bufs=1)
            qp = qp_cm.__enter__()
            # persistent qkv activations
            qT = qp.tile([P, BT], BF16)        # [2 heads x 64 d, t]
            kT = qp.tile([P, BT], BF16)
            # v_ext: [t-part, t-chunk, 2 x (64 d + ones col)]  (f32r)
            vx = qp.tile([P, NT128, 2 * (D + 1)], F32R)
            nc.gpsimd.memset(vx[:, :, D:D + 1].bitcast(F32), 1.0)
            nc.gpsimd.memset(vx[:, :, 2 * D + 1:2 * D + 2].bitcast(F32), 1.0)

            # ---------------- Phase A: embed -> xT -> q,k,v ----------------
            with (
                tc.tile_pool(name="pa", bufs=3) as pa,
                tc.tile_pool(name="paT", bufs=2) as paT,
                tc.tile_pool(name="paV", bufs=1) as paV,
                tc.tile_pool(name="pa_ps", bufs=3, space="PSUM") as pa_ps,
                tc.tile_pool(name="pa_ps2", bufs=3, space="PSUM") as pa_ps2,
            ):
                for a in range(NA):
                    xT = paT.tile([P, E // P, TCH], F32R, tag="xT")
                    for j in range(TCH // P):
                        t128 = a * (TCH // P) + j
                        idx = pa.tile([P, 1], dt.int32, tag="idx")
                        nc.sync.dma_start(idx[:], tokens[t128 * P:(t128 + 1) * P, :])
                        xg = pa.tile([P, E], F32, tag="xg")
                        nc.gpsimd.indirect_dma_start(
                            out=xg[:], out_offset=None, in_=tok_table[:],
                            in_offset=bass.IndirectOffsetOnAxis(ap=idx[:, :1], axis=0),
                        )
                        xp = pa.tile([P, E], F32, tag="xp")
                        tt = (t128 % (T // P)) * P
                        eng = nc.sync if j % 2 == 0 else nc.gpsimd
                        eng.dma_start(xp[:], pos[tt:tt + P, :])
                        xs = pa.tile([P, E], F32R, tag="xs")
                        nc.vector.tensor_tensor(out=xs[:], in0=xg[:], in1=xp[:], op=add_op)
                        for e8 in range(E // P):
                            pst = pa_ps.tile([P, P], F32R, tag="pst")
                            nc.tensor.transpose(pst[:], xs[:, e8 * P:(e8 + 1) * P], ident[:])
                            if e8 % 2 == 0:
                                nc.scalar.copy(xT[:, e8, j * P:(j + 1) * P], pst[:])
                            else:
                                nc.vector.tensor_copy(xT[:, e8, j * P:(j + 1) * P], pst[:])
                    # q,k (transposed layout, bf16) + bias on ACT
                    for wsb, b_sb, dst in ((wq_sb, bq_sb, qT), (wk_sb, bk_sb, kT)):
                        psq = pa_ps2.tile([P, TCH], F32, tag="psq")
                        for e8 in range(E // P):
                            nc.tensor.matmul(psq[:], wsb[:, e8, :], xT[:, e8, :],
                                             start=(e8 == 0), stop=(e8 == E // P - 1))
                        nc.scalar.activation(dst[:, a * TCH:(a + 1) * TCH], psq[:],
                                             Ident, bias=b_sb[:])
                    # vT then transpose to v natural (+ bias replicated)
                    psv = pa_ps2.tile([P, TCH], F32, tag="psq")
                    for e8 in range(E // P):
                        nc.tensor.matmul(psv[:], wv_sb[:, e8, :], xT[:, e8, :],
                                         start=(e8 == 0), stop=(e8 == E // P - 1))
                    vTc = paV.tile([P, TCH], F32R, tag="vTc")
                    nc.vector.tensor_copy(out=vTc[:], in_=psv[:])
                    for j in range(TCH // P):
                        t128 = a * (TCH // P) + j
                        psvt = pa_ps.tile([P, P], F32R, tag="pst")
                        nc.tensor.transpose(psvt[:], vTc[:, j * P:(j + 1) * P], ident[:])
                        for h in range(2):
                            nc.vector.tensor_tensor(
                                out=vx[:, t128, h * (D + 1):h * (D + 1) + D],
                                in0=psvt[:, h * D:(h + 1) * D],
                                in1=bvr_sb[:, h * D:(h + 1) * D], op=add_op)

            # ---------------- Phase B: attention ----------------
            with (
                tc.tile_pool(name="pb", bufs=6) as pb,
                tc.tile_pool(name="pb2", bufs=3) as pb2,
                tc.tile_pool(name="pb_ps", bufs=4, space="PSUM") as pb_ps,
                tc.tile_pool(name="pb_ps2", bufs=1, space="PSUM") as pb_ps2,
            ):
                for b in range(B):
                    for h in range(2):
                        qs = qT[h * D:(h + 1) * D, b * T:(b + 1) * T]
                        ks = kT[h * D:(h + 1) * D, b * T:(b + 1) * T]
                        psos = [pb_ps2.tile([D + 1, 512], F32, tag=f"pso{tq}",
                                            name=f"pso{tq}") for tq in range(NQ)]
                        all_pts = {}

                        def scores_pair(sb_):
                            pair = (sb_, sb_ + 1)
                            for tj in range(sb_ // 4, NQ):
                                for si in pair:
                                    if tj < si // 4:
                                        continue
                                    if si not in all_pts:
                                        all_pts[si] = pb.tile([P, T], F32R, tag="pt",
                                                              name=f"pt{si}")
                                    psw = pb_ps.tile([P, 512], F32, tag="psw",
                                                     name="psw")
                                    nc.tensor.matmul(
                                        psw[:], ks[:, si * P:(si + 1) * P],
                                        qs[:, tj * 512:(tj + 1) * 512],
                                        start=True, stop=True)
                                    nc.scalar.activation(
                                        all_pts[si][:, tj * 512:(tj + 1) * 512],
                                        psw[:], Exp)
                            for si in pair:
                                m = si % 4
                                toff = (si // 4) * 512
                                w = P * (m + 1)
                                nc.vector.tensor_tensor(
                                    out=all_pts[si][:, toff:toff + w],
                                    in0=all_pts[si][:, toff:toff + w],
                                    in1=mask_sb[:, m, :w], op=mult_op)

                        def ot_pair(sb_):
                            # oT accumulation (alternating lhsT between the pair)
                            for tq in range(sb_ // 4, NQ):
                                for si in (sb_, sb_ + 1):
                                    nc.tensor.matmul(
                                        psos[tq][:],
                                        vx[:, b * NS + si, h * (D + 1):(h + 1) * (D + 1)],
                                        all_pts[si][:, tq * 512:(tq + 1) * 512],
                                        start=(si == 0), stop=(si == 4 * tq + 3))
                            # tq = sb_//4 completes once pair (4tq+2, 4tq+3) done
                            if sb_ % 4 == 2:
                                tq = sb_ // 4
                                pso = psos[tq]
                                rr = pb2.tile([P, 512], F32R, tag="rr")
                                with nc.allow_low_precision(reason="1/Z feeds f32r bcast"):
                                    nc.vector.reciprocal(rr[D:D + 1, :], pso[D:D + 1, :])
                                psr = pb_ps.tile([D, 512], F32, tag="psw", name="psr")
                                nc.tensor.matmul(psr[:D], ones_sb[D:D + 1, :],
                                                 rr[D:D + 1, :], start=True, stop=True)
                                rrep = pb2.tile([D, 512], F32R, tag="rrep")
                                nc.vector.tensor_copy(rrep[:], psr[:D])
                                ot = pb2.tile([D, 512], F32R, tag="ot")
                                nc.vector.tensor_tensor(out=ot[:], in0=pso[:D, :],
                                                        in1=rrep[:], op=mult_op)
                                nc.sync.dma_start(
                                    ag_in[b][tq][h * D:(h + 1) * D, :], ot[:])
                                if h == 1:
                                    nc.gpsimd.collective_compute(
                                        "AllGather", mybir.AluOpType.bypass,
                                        replica_groups=[list(range(NC))],
                                        ins=[ag_in[b][tq].opt()],
                                        outs=[ag_out[b][tq].opt()],
                                    )

                        # software pipeline depth 2: scores two pairs ahead of oT
                        scores_pair(0)
                        scores_pair(2)
                        for sb_ in range(4, NS, 2):
                            scores_pair(sb_)
                            ot_pair(sb_ - 4)
                        ot_pair(NS - 4)
                        ot_pair(NS - 2)
                        if b == 0 and h == 0:
                            # Wo first half prefetch: sync queue, after phase
                            # A's latency-critical DMAs have drained
                            nc.sync.dma_start(woS1[:], woTp[:, :, :VS // 2])

            qp_cm.__exit__(None, None, None)

            # ================= Phase C: vocab projection + CE =================
            with (
                tc.tile_pool(name="pc", bufs=1) as pc,
                tc.tile_pool(name="pc2", bufs=2) as pc2,
                tc.tile_pool(name="pc3", bufs=3) as pc3,
                tc.tile_pool(name="pc_ps", bufs=1, space="PSUM") as pc_ps,
            ):
                bon = pc.tile([P, VS], F32)
                nc.gpsimd.dma_start(bon[:], bor[:])
                woS2 = pc.tile([P, E // P, VS // 2], F32R)
                nc.gpsimd.dma_start(woS2[:], woTp[:, :, VS // 2:])
                ce_acc = pc.tile([P, NT128, NV], F32)
                ce_sb = pc.tile([P, NT128], F32)
                for half in range(2):
                    woS, off = (woS1, 0) if half == 0 else (woS2, VS // 2)
                    for mI in range(NT128):
                        bI, mL = divmod(mI, T // P)
                        ag_ap = ag_out[bI][mL // 4].opt().rearrange(
                            "(k p) t -> p k t", p=P)
                        otm = pc2.tile([P, E // P, P], F32R, tag="otm")
                        nc.sync.dma_start(
                            otm[:], ag_ap[:, :, (mL % 4) * P:(mL % 4 + 1) * P])
                        psls = [pc_ps.tile([P, VCH], F32, tag=f"psl{n}",
                                           name=f"psl{n}")
                                for n in range(NV // 2)]
                        for nn in range(NV // 2):
                            n = half * (NV // 2) + nn
                            for e8 in range(E // P):
                                nc.tensor.matmul(
                                    psls[nn][:], otm[:, e8, :],
                                    woS[:, e8, n * VCH - off:(n + 1) * VCH - off],
                                    start=(e8 == 0), stop=(e8 == E // P - 1))
                        for nn in range(NV // 2):
                            n = half * (NV // 2) + nn
                            lsb = pc3.tile([P, VCH], F32, tag="lsb")
                            nc.vector.tensor_tensor(out=lsb[:], in0=psls[nn][:],
                                                    in1=bon[:, n * VCH:(n + 1) * VCH],
                                                    op=add_op)
                            nc.sync.dma_start(
                                logits[mI * P:(mI + 1) * P, n * VCH:(n + 1) * VCH],
                                lsb[:])
                            esc = pc3.tile([P, VCH], F32, tag="esc")
                            nc.scalar.activation(esc[:], lsb[:], Exp,
                                                 accum_out=ce_acc[:, mI, n:n + 1])
                for mI in range(NT128):
                    nc.vector.reduce_sum(out=ce_sb[:, mI:mI + 1], in_=ce_acc[:, mI, :],
                                         axis=mybir.AxisListType.X)
                nc.sync.dma_start(ce[:], ce_sb[:])
            pcW_cm.__exit__(None, None, None)

    nc.compile()
    return nc


def _get_nc():
    if "nc" not in _CACHE:
        _CACHE["nc"] = _build()
    return _CACHE["nc"]


def kernel(tokens, targets, tok_table, pos_emb, Wq, bq, Wk, bk, Wv, bv, Wo, bo):
    from concourse.bass_utils import run_bass_kernel_spmd

    tokens = np.asarray(tokens)
    targets = np.asarray(targets)
    tok_table = np.ascontiguousarray(np.asarray(tok_table, np.float32))
    pos_emb = np.ascontiguousarray(np.asarray(pos_emb, np.float32))
    Wq = np.asarray(Wq, np.float32)
    Wk = np.asarray(Wk, np.float32)
    Wv = np.asarray(Wv, np.float32)
    bq = np.asarray(bq, np.float32)
    bk = np.asarray(bk, np.float32)
    bv = np.asarray(bv, np.float32)
    Wo = np.asarray(Wo, np.float32)
    bo = np.asarray(bo, np.float32)

    tok_i = tokens.reshape(BT, 1).astype(np.int32)
    scale = np.float32(1.0 / np.sqrt(D))  # folded into Wv/bv (quirk: post-softmax)

    # host-side layout prep (pure reshapes/transposes of weights)
    # mask4[m][sp, c] = 1 iff c >= 128*m + sp   (prefix zeros + shifted triu)
    cidx = np.arange(512)[None, :]
    spidx = np.arange(P)[:, None]
    mask4 = np.stack([(cidx >= 128 * m + spidx).astype(np.float32)
                      for m in range(4)], axis=1)         # [128, 4, 512]
    mask4 = np.ascontiguousarray(mask4)

    def pk(w):  # [E, X] -> [128, 8, X] contiguous (partition-major K-subtiles)
        return np.ascontiguousarray(w.reshape(E // P, P, -1).transpose(1, 0, 2))

    in_maps = []
    for c in range(NC):
        h0 = HL * c
        wq_c = pk(Wq[h0:h0 + HL].reshape(HL * D, E).T)
        wk_c = pk(Wk[h0:h0 + HL].reshape(HL * D, E).T)
        wv_c = pk((Wv[h0:h0 + HL] * scale).reshape(HL * D, E).T)
        bq_c = bq[h0:h0 + HL].reshape(P, 1).copy()
        bk_c = bk[h0:h0 + HL].reshape(P, 1).copy()
        bv_c = np.broadcast_to((bv[h0:h0 + HL] * scale).reshape(1, P), (P, P)).copy()
        wo_c = pk(Wo[c * VS:(c + 1) * VS].T)              # [128, 8, VS]
        bo_c = np.broadcast_to(bo[c * VS:(c + 1) * VS][None, :], (P, VS)).copy()
        in_maps.append({
            "tokens": tok_i, "tok_table": tok_table, "pos": pos_emb,
            "wqT": wq_c, "wkT": wk_c, "wvT": wv_c,
            "bqc": bq_c, "bkc": bk_c, "bvr": bv_c,
            "woTp": wo_c, "bor": bo_c, "mask4": mask4,
            "identp": np.eye(P, dtype=np.float32),
            "onesp": np.ones((P, D), np.float32),
        })

    nc = _get_nc()
    res = run_bass_kernel_spmd(nc, in_maps, core_ids=list(range(NC)), **_RUN_KW)
    global LAST_RESULT
    LAST_RESULT = res

    logits_full = np.empty((BT, V), np.float32)
    S = np.zeros(BT, np.float64)
    for c in range(NC):
        r = res.results[c]
        logits_full[:, c * VS:(c + 1) * VS] = r["logits"]
        S += r["ce"].T.reshape(BT).astype(np.float64)

    tl = targets.reshape(BT).astype(np.int64)
    l_tgt = logits_full[np.arange(BT), tl].astype(np.float64)
    loss = np.float32(np.mean(np.log(S) - l_tgt))
    return logits_full, loss
